# revision 3
# baseline (speedup 1.0000x reference)
"""Trainium2 Bass kernel for nn_DirectedHyperConvLayer (GNN message passing).

Self-contained: accepts FULL inputs, shards across 8 NeuronCores internally,
returns the FULL [50000, 64] float32 output.

Sharding: each core owns a contiguous block of destination rows; the host
routes/sorts edges by destination row, pads them into 128-edge tiles grouped
by 128-row destination windows, and splits each window's edges into low/high
passes so source indices fit dma_gather's int16 limit. On device, combined
[raw_bf16 | normalized_bf16] rows are fetched with batched dma_gather calls.
Per-edge cosine weights: a host-supplied f8 one-hot (spr, [dest-row, edge])
selects the destination-window embedding block through the tensor engine;
a fused tensor_tensor_reduce computes 21 + hat_dst . hat_src per edge; the
weight wv = dot1 * (0.05*val) is applied by the Scalar engine onto a second
host-supplied f8 one-hot (s8, [edge, dest-row]) producing a bf16 weighted
one-hot, which the tensor engine multiplies against raw gathered rows to
segment-sum messages into PSUM per window. Stage 1 results (raw bf16 only)
are AllGathered and merged into the stage-1 table's raw half for stage 2.
"""


import math
from dataclasses import dataclass

import numpy as np
import ml_dtypes

import concourse.bass as bass
import concourse.bacc as bacc
import concourse.mybir as mybir
import concourse.tile as tile

F32 = mybir.dt.float32
BF16 = mybir.dt.bfloat16
F8 = mybir.dt.float8e4
I16 = mybir.dt.int16
I32 = mybir.dt.int32
NP_F8 = mybir.dt.np(F8)
P = 128
TB = 8  # tiles per gather call (1024 idx)
HB = 8  # tiles per compute half-batch
ALPHA = 0.1


@dataclass
class Config:
    n_nodes: int = 50000
    d: int = 64
    n_cores: int = 8
    rpc: int = 6272  # rows per core (multiple of 128)
    split: int = 32768  # low/high gather split (<= 32768)

    @property
    def nw(self):
        return self.rpc // P

    @property
    def n_pad(self):
        return self.n_cores * self.rpc

    @property
    def nblk(self):
        return self.n_pad // P


@dataclass
class StageSched:
    T: np.ndarray  # [2, nw] tiles per (pass, window)
    n_tiles: tuple  # (low, high) tile counts (each % TB == 0)

    @property
    def total_tiles(self):
        return int(self.n_tiles[0] + self.n_tiles[1])

    def tile_windows(self):
        """list over global tile index -> (pass, w, j_in_window, first, last)"""
        out = []
        for p in range(2):
            for w in range(self.T.shape[1]):
                Tw = int(self.T[p, w])
                for j in range(Tw):
                    out.append((p, w, j, j == 0, j == Tw - 1))
        return out


def route_edges(cfg: Config, edge_index, edge_val):
    """Returns (sched, per_core list of dicts with idx16/valp/spr/s8)."""
    r0 = np.asarray(edge_index[0], dtype=np.int64)
    r1 = np.asarray(edge_index[1], dtype=np.int64)
    val = np.asarray(edge_val, dtype=np.float32)
    E = r0.shape[0]
    nc_, nw = cfg.n_cores, cfg.nw

    k = r0 // cfg.rpc
    w = (r0 % cfg.rpc) // P
    dloc = r0 % P
    hi = (r1 >= cfg.split).astype(np.int64)
    gid = (k * 2 + hi) * nw + w

    counts = np.bincount(gid, minlength=nc_ * 2 * nw).reshape(nc_, 2, nw)
    T = np.ceil(counts.max(axis=0) / P).astype(np.int64)  # [2, nw]
    # pad each pass's tile total to a multiple of TB
    for pss in range(2):
        T[pss, nw - 1] += (-int(T[pss].sum())) % TB
    nt_low, nt_high = int(T[0].sum()), int(T[1].sum())
    n_tiles = nt_low + nt_high
    # global tile base per (pass, w)
    tbase = np.zeros((2, nw), dtype=np.int64)
    tbase[0] = np.cumsum(T[0]) - T[0]
    tbase[1] = nt_low + np.cumsum(T[1]) - T[1]

    # slot within (k, hi, w) group
    order = np.argsort(gid, kind="stable")
    sorted_gid = gid[order]
    starts = np.searchsorted(sorted_gid, np.arange(nc_ * 2 * nw))
    ranks = np.empty(E, dtype=np.int64)
    ranks[order] = np.arange(E) - starts[sorted_gid]

    tile_g = tbase[hi, w] + ranks // P  # global tile per edge
    pos = ranks % P
    idx_val = (r1 - hi * cfg.split).astype(np.int16)

    ncalls = n_tiles // TB
    per_core = []
    for kk in range(nc_):
        m = k == kk
        tg, pg = tile_g[m], pos[m]
        slots = tg * P + pg
        idx_flat = np.zeros(n_tiles * P, dtype=np.int16)
        valp_flat = np.zeros(n_tiles * P, dtype=np.float32)
        idx_flat[slots] = idx_val[m]
        valp_flat[slots] = 0.05 * val[m]
        # idx16 wrapped: [128, ncalls*(TB*P//16)]
        cw = TB * P // 16
        iw = idx_flat.reshape(ncalls, cw, 16)
        iw = np.transpose(iw, (2, 0, 1)).reshape(16, ncalls * cw)
        idx16 = np.tile(iw, (8, 1))
        # valp [128, n_tiles] (partition p, tile t)
        valp2d = valp_flat.reshape(n_tiles, P).T.copy()
        # spr [128(d), n_tiles*128(e)] f8 ; s8 [128(e), n_tiles*128(d)] f8
        dl = dloc[m]
        spr = np.zeros((P, n_tiles * P), dtype=NP_F8)
        spr[dl, slots] = NP_F8(1.0)
        s8 = np.zeros((P, n_tiles * P), dtype=NP_F8)
        s8[pg, tg * P + dl] = NP_F8(1.0)
        per_core.append({"idx": idx16, "valp": valp2d, "spr": spr, "s8": s8})
    return StageSched(T=T, n_tiles=(nt_low, nt_high)), per_core


def _emit_stage(
    tc, cfg, sched: StageSched, pools, consts, table, idx_t, valp_t,
    spr_dram, s8_dram, out_close,
):
    """Emit one spmm stage. out_close(w, psum_ap, acc_ap, has_low) writes the
    finished window."""
    nc = tc.nc
    nw = cfg.nw
    d = cfg.d
    gp, aalp, paccp, dvep, s8wp, accp, sprp, s8p = (
        pools["g"], pools["aal"], pools["pacc"], pools["dve"], pools["s8w"],
        pools["acc"], pools["spr"], pools["s8"],
    )
    strip = consts["strip"]

    acc = accp.tile([P, nw * d], F32, tag="acc")
    nc.vector.memset(acc[:], 0.0)

    tw = sched.tile_windows()
    n_tiles = sched.total_tiles
    assert n_tiles % TB == 0
    ncalls = n_tiles // TB
    nt_low = sched.n_tiles[0]
    cw = TB * P // 16  # idx cols per call

    # spr/s8 window tiles, streamed per (pass, w)
    spr_tiles = {}
    s8_tiles = {}
    st0 = 0
    for pss in range(2):
        for w in range(nw):
            Tw = int(sched.T[pss, w])
            if Tw == 0:
                continue
            st = sprp.tile([P, Tw * P], F8, tag="spr", name="spr")
            nc.sync.dma_start(st[:], spr_dram[:, st0 * P : (st0 + Tw) * P])
            s8t = s8p.tile([P, Tw * P], F8, tag="s8", name="s8")
            nc.sync.dma_start(s8t[:], s8_dram[:, st0 * P : (st0 + Tw) * P])
            spr_tiles[(pss, w)] = (st, st0)
            s8_tiles[(pss, w)] = s8t
            st0 += Tw

    win_psum = {}
    for c in range(ncalls):
        pss = 0 if c * TB < nt_low else 1
        tab = table[0 : cfg.split, :] if pss == 0 else table[cfg.split : cfg.n_pad, :]
        g = gp.tile([P, TB, 2 * d], BF16, tag="g")
        nc.gpsimd.dma_gather(
            out_ap=g[:],
            in_ap=tab,
            idxs_ap=idx_t[:, c * cw : (c + 1) * cw],
            num_idxs=TB * P,
            num_idxs_reg=TB * P,
            elem_size=2 * d,
            queue_num=c % 4,
            single_packet=True,
        )
        t0 = c * TB  # first tile of this batch (TB == HB)
        # Aal matmuls (per tile)
        aal = aalp.tile([P, HB, d], F32, space="PSUM", tag="aal")
        for sl in range(HB):
            t = t0 + sl
            pss_t, w, _, _, _ = tw[t]
            st, st_t0 = spr_tiles[(pss_t, w)]
            jj = t - st_t0
            nc.tensor.matmul(
                out=aal[:, sl, :],
                lhsT=st[:, jj * P : (jj + 1) * P],
                rhs=strip[:, w * d : (w + 1) * d],
                start=True,
                stop=True,
            )
        # dot per edge: dot1[e] = 21 + sum_f aal[e,f] * ghat[e,f]
        prod = dvep.tile([P, HB, d], F32, tag="prod")
        nc.vector.tensor_tensor(
            out=prod[:], in0=aal[:], in1=g[:, :, d : 2 * d],
            op=mybir.AluOpType.mult,
        )
        dot1 = dvep.tile([P, HB], F32, tag="dot1")
        nc.vector.tensor_reduce(
            out=dot1[:], in_=prod[:], op=mybir.AluOpType.add,
            axis=mybir.AxisListType.X,
        )
        nc.vector.tensor_scalar_add(dot1[:], dot1[:], 21.0)
        # wv = dot1 * (0.05*val)  (== val * (1.05 + 0.05*sim_dot))
        wv = dvep.tile([P, HB], F32, tag="wv")
        nc.vector.tensor_tensor(
            out=wv[:], in0=dot1[:], in1=valp_t[:, t0 : t0 + HB],
            op=mybir.AluOpType.mult,
        )
        # weighted one-hot on scalar engine: s8w[e, dd] = wv[e] * s8[e, dd]
        s8w = s8wp.tile([P, HB * P], BF16, tag="s8w")
        for sl in range(HB):
            t = t0 + sl
            pss_t, w, _, _, _ = tw[t]
            s8t = s8_tiles[(pss_t, w)]
            st_t0 = spr_tiles[(pss_t, w)][1]
            jj = t - st_t0
            nc.scalar.activation(
                out=s8w[:, sl * P : (sl + 1) * P],
                in_=s8t[:, jj * P : (jj + 1) * P],
                func=mybir.ActivationFunctionType.Copy,
                scale=wv[:, sl : sl + 1],
            )
        # scatter matmuls
        for sl in range(HB):
            t = t0 + sl
            pss_t, w, j, first, last = tw[t]
            key = (pss_t, w)
            if key not in win_psum:
                win_psum[key] = paccp.tile(
                    [P, d], F32, space="PSUM", tag="pacc", name="pacc"
                )
            pw = win_psum[key]
            nc.tensor.matmul(
                out=pw[:],
                lhsT=s8w[:, sl * P : (sl + 1) * P],
                rhs=g[:, sl, 0:d],
                start=first,
                stop=last,
            )
            if last:
                if pss_t == 0:
                    nc.scalar.copy(out=acc[:, w * d : (w + 1) * d], in_=pw[:])
                else:
                    has_low = sched.T[0, w] > 0
                    out_close(w, pw[:], acc[:, w * d : (w + 1) * d], has_low)
                del win_psum[key]
    # windows with no high-pass tiles: close from acc only
    for w in range(nw):
        if sched.T[1, w] == 0:
            out_close(w, None, acc[:, w * d : (w + 1) * d], sched.T[0, w] > 0)


def build_kernel(cfg: Config, sched1: StageSched, sched2: StageSched):
    nc = bacc.Bacc(
        "TRN2",
        target_bir_lowering=False,
        debug=False,
        enable_asserts=False,
        num_devices=cfg.n_cores,
        num_swdge_queues=4,
    )
    d = cfg.d
    embs = nc.dram_tensor("embs", [cfg.n_nodes, d], F32, kind="ExternalInput")
    sio = {}
    for s, sch in (("s1", sched1), ("s2", sched2)):
        nt = sch.total_tiles
        sio[s] = {
            "idx": nc.dram_tensor(f"{s}_idx", [P, (nt // TB) * (TB * P // 16)], I16, kind="ExternalInput"),
            "valp": nc.dram_tensor(f"{s}_valp", [P, nt], F32, kind="ExternalInput"),
            "spr": nc.dram_tensor(f"{s}_spr", [P, nt * P], F8, kind="ExternalInput"),
            "s8": nc.dram_tensor(f"{s}_s8", [P, nt * P], F8, kind="ExternalInput"),
        }
    out = nc.dram_tensor("out", [cfg.rpc, d], F32, kind="ExternalOutput")
    aug1 = nc.dram_tensor("aug1", [cfg.n_pad, 2 * d], BF16)
    msgtar_local = nc.dram_tensor("mt_loc", [cfg.rpc, d], BF16)
    msgtar_full = nc.dram_tensor("mt_full", [cfg.n_pad, d], BF16)

    with tile.TileContext(nc) as tc:
        cpool = tc.alloc_tile_pool(name="const", bufs=1)
        # constants
        strip = cpool.tile([P, cfg.nw * d], F8)

        # ---------------- prologue: build aug1 + hat strip ----------------
        nblk = cfg.nblk
        CB = 56 if nblk % 56 == 0 else max(
            c for c in range(1, 65) if nblk % c == 0
        )
        nchunks = nblk // CB
        with tc.tile_pool(name="pro", bufs=2) as pro, tc.tile_pool(
            name="pro1", bufs=1
        ) as pro1:
            hatf8 = pro1.tile([P, nblk * d], F8)
            for c in range(nchunks):
                r0 = c * CB * P  # first natural row of chunk
                rows = CB * P
                raw = pro.tile([P, CB, d], F32, tag="praw")
                # rows r0 + t*128 + p; valid rows < n_nodes
                valid = min(rows, max(0, cfg.n_nodes - r0))
                full_t = valid // P
                if valid < rows:
                    nc.vector.memset(raw[:], 0.0)
                if full_t > 0:
                    nc.sync.dma_start(
                        out=raw[:, 0:full_t, :],
                        in_=embs[r0 : r0 + full_t * P].rearrange(
                            "(t p) f -> p t f", p=P
                        ),
                    )
                rem = valid - full_t * P
                if rem > 0:
                    nc.sync.dma_start(
                        out=raw[0:rem, full_t, :],
                        in_=embs[r0 + full_t * P : r0 + valid].rearrange(
                            "(t p) f -> p t f", p=rem
                        )[0:rem, 0, :],
                    )
                tmp = pro.tile([P, CB, d], F32, tag="ptmp")
                nc.vector.tensor_tensor(
                    out=tmp[:], in0=raw[:], in1=raw[:], op=mybir.AluOpType.mult
                )
                nrm2 = pro.tile([P, CB], F32, tag="pnrm")
                nc.vector.tensor_reduce(
                    out=nrm2[:], in_=tmp[:], op=mybir.AluOpType.add,
                    axis=mybir.AxisListType.X,
                )
                nc.vector.tensor_scalar_add(nrm2[:], nrm2[:], 1e-12)
                rec = pro.tile([P, CB], F32, tag="prec")
                nc.vector.reciprocal(rec[:], nrm2[:])
                invn = pro.tile([P, CB], F32, tag="pinv")
                nc.scalar.sqrt(invn[:], rec[:])
                comb = pro.tile([P, CB, 2 * d], BF16, tag="pcomb")
                nc.vector.tensor_copy(out=comb[:, :, 0:d], in_=raw[:])
                nc.vector.tensor_tensor(
                    out=comb[:, :, d : 2 * d], in0=raw[:],
                    in1=invn[:].to_broadcast([P, CB, d]), op=mybir.AluOpType.mult,
                )
                nc.vector.tensor_copy(
                    out=hatf8[:, c * CB * d : (c + 1) * CB * d],
                    in_=comb[:, :, d : 2 * d],
                )
                nc.sync.dma_start(
                    out=aug1[r0 : r0 + rows].rearrange("(t p) f -> p t f", p=P),
                    in_=comb[:],
                )
            # per-core window strip: global window W = pid*nw + w
            pid = nc.vector.partition_id()
            nc.vector.tensor_copy(
                out=strip[:],
                in_=hatf8[:, bass.ds(pid * (cfg.nw * d), cfg.nw * d)],
            )

        # ---------------- stage pools ----------------
        pools = {
            "g": tc.alloc_tile_pool(name="g", bufs=12),
            "aal": tc.alloc_tile_pool(name="aal", bufs=4, space="PSUM"),
            "pacc": tc.alloc_tile_pool(name="pacc", bufs=3, space="PSUM"),
            "spr": tc.alloc_tile_pool(name="sprp", bufs=3),
            "s8": tc.alloc_tile_pool(name="s8p", bufs=3),
            "dve": tc.alloc_tile_pool(name="dve", bufs=8),
            "s8w": tc.alloc_tile_pool(name="s8w", bufs=6),
            "acc": tc.alloc_tile_pool(name="accp", bufs=1),
            "io": tc.alloc_tile_pool(name="iop", bufs=1),
            "wout": tc.alloc_tile_pool(name="wout", bufs=4),
        }
        consts = {"strip": strip}

        def load_stage_meta(s):
            io = pools["io"]
            t = sio[s]
            nt = t["valp"].shape[1]
            idx_t = io.tile([P, t["idx"].shape[1]], I16, tag=f"{s}i", name=f"{s}i")
            nc.sync.dma_start(idx_t[:], t["idx"][:, :])
            valp_t = io.tile([P, nt], F32, tag=f"{s}v", name=f"{s}v")
            nc.sync.dma_start(valp_t[:], t["valp"][:, :])
            return idx_t, valp_t, t["spr"], t["s8"]

        # ---------------- stage 1 (tar) ----------------
        idx1, valp1, spr1, s81 = load_stage_meta("s1")

        def close1(w, psum_ap, acc_ap, has_low):
            ot = pools["wout"].tile([P, d], BF16, tag="wo1")
            if psum_ap is None:
                nc.scalar.copy(out=ot[:], in_=acc_ap)
            elif has_low:
                nc.vector.tensor_tensor(
                    out=ot[:], in0=acc_ap, in1=psum_ap, op=mybir.AluOpType.add
                )
            else:
                nc.scalar.copy(out=ot[:], in_=psum_ap)
            nc.sync.dma_start(
                out=msgtar_local[w * P : (w + 1) * P, :], in_=ot[:]
            )

        _emit_stage(
            tc, cfg, sched1, pools, consts, aug1[:], idx1, valp1, spr1, s81,
            close1,
        )

        # ---------------- allgather (raw msg_tar only) ----------------
        nc.gpsimd.collective_compute(
            "AllGather",
            mybir.AluOpType.bypass,
            replica_groups=[list(range(cfg.n_cores))],
            ins=[msgtar_local[:].opt()],
            outs=[msgtar_full[:].opt()],
        )
        # merge gathered raw into aug1's raw half (hat half already valid)
        nc.sync.dma_start(out=aug1[:, 0:d], in_=msgtar_full[:, :])

        # ---------------- stage 2 (src) ----------------
        idx2, valp2, spr2, s82 = load_stage_meta("s2")

        def close2(w, psum_ap, acc_ap, has_low):
            ot = pools["wout"].tile([P, d], F32, tag="wo2")
            if psum_ap is None:
                nc.scalar.copy(out=ot[:], in_=acc_ap)
            elif has_low:
                nc.vector.tensor_tensor(
                    out=ot[:], in0=acc_ap, in1=psum_ap, op=mybir.AluOpType.add
                )
            else:
                nc.scalar.copy(out=ot[:], in_=psum_ap)
            nc.sync.dma_start(out=out[w * P : (w + 1) * P, :], in_=ot[:])

        _emit_stage(
            tc, cfg, sched2, pools, consts, aug1[:], idx2, valp2, spr2, s82,
            close2,
        )

        for p in reversed(list(pools.values())):
            p.release()
        cpool.release()

    nc.compile()
    return nc


def prepare(cfg: Config, inputs):
    """inputs: dict with pois_embs, src_edge_index, src_edge_val, tar_*."""
    sched1, meta1 = route_edges(cfg, inputs["tar_edge_index"], inputs["tar_edge_val"])
    sched2, meta2 = route_edges(cfg, inputs["src_edge_index"], inputs["src_edge_val"])
    embs = np.asarray(inputs["pois_embs"], dtype=np.float32)
    in_maps = []
    for k in range(cfg.n_cores):
        in_maps.append(
            {
                "embs": embs,
                "s1_idx": meta1[k]["idx"], "s1_valp": meta1[k]["valp"],
                "s1_spr": meta1[k]["spr"], "s1_s8": meta1[k]["s8"],
                "s2_idx": meta2[k]["idx"], "s2_valp": meta2[k]["valp"],
                "s2_spr": meta2[k]["spr"], "s2_s8": meta2[k]["s8"],
            }
        )
    return sched1, sched2, in_maps


def assemble_output(cfg: Config, results):
    out = np.zeros((cfg.n_nodes, cfg.d), dtype=np.float32)
    for k, r in enumerate(results):
        lo = k * cfg.rpc
        hi = min(lo + cfg.rpc, cfg.n_nodes)
        if hi > lo:
            out[lo:hi] = r["out"][0 : hi - lo]
    return out


_CACHE = {}


def kernel(**inputs):
    import concourse.bass_utils as bass_utils

    cfg = Config()
    sched1, sched2, in_maps = prepare(cfg, inputs)
    key = (sched1.n_tiles, sched2.n_tiles, tuple(sched1.T.ravel()), tuple(sched2.T.ravel()))
    nc = _CACHE.get(key)
    if nc is None:
        nc = build_kernel(cfg, sched1, sched2)
        _CACHE[key] = nc
    res = bass_utils.run_bass_kernel_spmd(
        nc, in_maps, core_ids=list(range(cfg.n_cores)), trace=False
    )
    out = assemble_output(cfg, res.results)
    return out.astype(np.float32, copy=False)


# revision 5
# speedup vs baseline: 1.8846x; 1.8846x over previous
"""Trainium2 Bass kernel for nn_DirectedHyperConvLayer (GNN message passing).

Self-contained: accepts FULL inputs, shards across 8 NeuronCores internally,
returns the FULL [50000, 64] float32 output.

Sharding: each core owns a contiguous block of destination rows; the host
routes/sorts edges by destination row, pads them into 128-edge tiles grouped
by 128-row destination windows, and splits each window's edges into low/high
passes so source indices fit dma_gather's int16 limit. On device, combined
[raw_bf16 | normalized_bf16] rows are fetched with batched dma_gather calls.
Per-edge cosine weights: a host-supplied f8 one-hot (spr, [dest-row, edge])
selects the destination-window embedding block through the tensor engine;
a fused tensor_tensor_reduce computes 21 + hat_dst . hat_src per edge; the
weight wv = dot1 * (0.05*val) is applied by the Scalar engine onto a second
host-supplied f8 one-hot (s8, [edge, dest-row]) producing a bf16 weighted
one-hot, which the tensor engine multiplies against raw gathered rows to
segment-sum messages into PSUM per window. Stage 1 results (raw bf16 only)
are AllGathered and merged into the stage-1 table's raw half for stage 2.
"""


import math
from dataclasses import dataclass

import numpy as np
import ml_dtypes

import concourse.bass as bass
import concourse.bacc as bacc
import concourse.mybir as mybir
import concourse.tile as tile

F32 = mybir.dt.float32
BF16 = mybir.dt.bfloat16
F8 = mybir.dt.float8e4
I16 = mybir.dt.int16
I32 = mybir.dt.int32
NP_F8 = mybir.dt.np(F8)
P = 128
TB = 8  # tiles per gather call (1024 idx)
HB = 8  # tiles per compute half-batch
ALPHA = 0.1


@dataclass
class Config:
    n_nodes: int = 50000
    d: int = 64
    n_cores: int = 8
    rpc: int = 6272  # rows per core (multiple of 128)
    split: int = 32768  # low/high gather split (<= 32768)

    @property
    def nw(self):
        return self.rpc // P

    @property
    def n_pad(self):
        return self.n_cores * self.rpc

    @property
    def nblk(self):
        return self.n_pad // P


@dataclass
class StageSched:
    T: np.ndarray  # [2, nw] tiles per (pass, window)
    n_tiles: tuple  # (low, high) tile counts (each % TB == 0)

    @property
    def total_tiles(self):
        return int(self.n_tiles[0] + self.n_tiles[1])

    def tile_windows(self):
        """list over global tile index -> (pass, w, j_in_window, first, last)"""
        out = []
        for p in range(2):
            for w in range(self.T.shape[1]):
                Tw = int(self.T[p, w])
                for j in range(Tw):
                    out.append((p, w, j, j == 0, j == Tw - 1))
        return out


def route_edges(cfg: Config, edge_index, edge_val):
    """Returns (sched, per_core list of dicts with idx16/valp/spr/s8)."""
    r0 = np.asarray(edge_index[0], dtype=np.int64)
    r1 = np.asarray(edge_index[1], dtype=np.int64)
    val = np.asarray(edge_val, dtype=np.float32)
    E = r0.shape[0]
    nc_, nw = cfg.n_cores, cfg.nw

    k = r0 // cfg.rpc
    w = (r0 % cfg.rpc) // P
    dloc = r0 % P
    hi = (r1 >= cfg.split).astype(np.int64)
    gid = (k * 2 + hi) * nw + w

    counts = np.bincount(gid, minlength=nc_ * 2 * nw).reshape(nc_, 2, nw)
    T = np.ceil(counts.max(axis=0) / P).astype(np.int64)  # [2, nw]
    # pad each pass's tile total to a multiple of TB
    for pss in range(2):
        T[pss, nw - 1] += (-int(T[pss].sum())) % TB
    nt_low, nt_high = int(T[0].sum()), int(T[1].sum())
    n_tiles = nt_low + nt_high
    # global tile base per (pass, w)
    tbase = np.zeros((2, nw), dtype=np.int64)
    tbase[0] = np.cumsum(T[0]) - T[0]
    tbase[1] = nt_low + np.cumsum(T[1]) - T[1]

    # slot within (k, hi, w) group
    order = np.argsort(gid, kind="stable")
    sorted_gid = gid[order]
    starts = np.searchsorted(sorted_gid, np.arange(nc_ * 2 * nw))
    ranks = np.empty(E, dtype=np.int64)
    ranks[order] = np.arange(E) - starts[sorted_gid]

    tile_g = tbase[hi, w] + ranks // P  # global tile per edge
    pos = ranks % P
    idx_val = (r1 - hi * cfg.split).astype(np.int16)

    ncalls = n_tiles // TB
    per_core = []
    for kk in range(nc_):
        m = k == kk
        tg, pg = tile_g[m], pos[m]
        slots = tg * P + pg
        idx_flat = np.zeros(n_tiles * P, dtype=np.int16)
        valp_flat = np.zeros(n_tiles * P, dtype=np.float32)
        idx_flat[slots] = idx_val[m]
        valp_flat[slots] = 0.05 * val[m]
        # idx16 wrapped: [128, ncalls*(TB*P//16)]
        cw = TB * P // 16
        iw = idx_flat.reshape(ncalls, cw, 16)
        iw = np.transpose(iw, (2, 0, 1)).reshape(16, ncalls * cw)
        idx16 = np.tile(iw, (8, 1))
        # valp [128, n_tiles] (partition p, tile t)
        valp2d = valp_flat.reshape(n_tiles, P).T.copy()
        # spr [128(d), n_tiles*128(e)] f8 ; s8 [128(e), n_tiles*128(d)] f8
        dl = dloc[m]
        spr = np.zeros((P, n_tiles * P), dtype=NP_F8)
        spr[dl, slots] = NP_F8(1.0)
        s8 = np.zeros((P, n_tiles * P), dtype=NP_F8)
        s8[pg, tg * P + dl] = NP_F8(1.0)
        per_core.append({"idx": idx16, "valp": valp2d, "spr": spr, "s8": s8})
    return StageSched(T=T, n_tiles=(nt_low, nt_high)), per_core


def _emit_stage(
    tc, cfg, sched: StageSched, pools, consts, table, idx_t, valp_t,
    spr_dram, s8_dram, out_close,
):
    """Emit one spmm stage. out_close(w, psum_ap, acc_ap, has_low) writes the
    finished window.

    The scatter matmuls for call c are emitted during call c+1 (software
    pipelining) so the tensor engine's in-order queue does not stall the
    aal matmuls of later calls behind the dot/weight chain of call c."""
    nc = tc.nc
    nw = cfg.nw
    d = cfg.d
    gp, aalp, paccp, dvep, msgp, accp, sprp, s8p = (
        pools["g"], pools["aal"], pools["pacc"], pools["dve"], pools["msg"],
        pools["acc"], pools["spr"], pools["s8"],
    )
    strip = consts["strip"]

    acc = accp.tile([P, nw * d], F32, tag="acc")
    nc.vector.memset(acc[:], 0.0)

    tw = sched.tile_windows()
    n_tiles = sched.total_tiles
    assert n_tiles % TB == 0
    ncalls = n_tiles // TB
    nt_low = sched.n_tiles[0]
    cw = TB * P // 16  # idx cols per call

    # spr/s8 window tiles, streamed per (pass, w)
    spr_tiles = {}
    s8_tiles = {}
    st0 = 0
    for pss in range(2):
        for w in range(nw):
            Tw = int(sched.T[pss, w])
            if Tw == 0:
                continue
            st = sprp.tile([P, Tw * P], F8, tag="spr", name="spr")
            nc.sync.dma_start(st[:], spr_dram[:, st0 * P : (st0 + Tw) * P])
            s8t = s8p.tile([P, Tw * P], F8, tag="s8", name="s8")
            nc.sync.dma_start(s8t[:], s8_dram[:, st0 * P : (st0 + Tw) * P])
            spr_tiles[(pss, w)] = (st, st0)
            s8_tiles[(pss, w)] = s8t
            st0 += Tw

    win_psum = {}

    def emit_scatter(c, msgs):
        t0 = c * TB
        for sl in range(HB):
            t = t0 + sl
            pss_t, w, j, first, last = tw[t]
            key = (pss_t, w)
            if key not in win_psum:
                win_psum[key] = paccp.tile(
                    [P, d], F32, space="PSUM", tag="pacc", name="pacc"
                )
            pw = win_psum[key]
            s8t = s8_tiles[key]
            st_t0 = spr_tiles[key][1]
            jj = t - st_t0
            nc.tensor.matmul(
                out=pw[:],
                lhsT=s8t[:, jj * P : (jj + 1) * P],
                rhs=msgs[:, sl, :],
                start=first,
                stop=last,
            )
            if last:
                if pss_t == 0:
                    nc.scalar.copy(out=acc[:, w * d : (w + 1) * d], in_=pw[:])
                else:
                    has_low = sched.T[0, w] > 0
                    out_close(w, pw[:], acc[:, w * d : (w + 1) * d], has_low)
                del win_psum[key]

    pend = None  # (c, msgs) awaiting scatter
    for c in range(ncalls):
        pss = 0 if c * TB < nt_low else 1
        tab = table[0 : cfg.split, :] if pss == 0 else table[cfg.split : cfg.n_pad, :]
        g = gp.tile([P, TB, 2 * d], BF16, tag="g")
        nc.gpsimd.dma_gather(
            out_ap=g[:],
            in_ap=tab,
            idxs_ap=idx_t[:, c * cw : (c + 1) * cw],
            num_idxs=TB * P,
            num_idxs_reg=TB * P,
            elem_size=2 * d,
            queue_num=c % 4,
            single_packet=True,
        )
        t0 = c * TB  # first tile of this batch (TB == HB)
        # Aal matmuls (per tile)
        aal = aalp.tile([P, HB, d], F32, space="PSUM", tag="aal")
        for sl in range(HB):
            t = t0 + sl
            pss_t, w, _, _, _ = tw[t]
            st, st_t0 = spr_tiles[(pss_t, w)]
            jj = t - st_t0
            nc.tensor.matmul(
                out=aal[:, sl, :],
                lhsT=st[:, jj * P : (jj + 1) * P],
                rhs=strip[:, w * d : (w + 1) * d],
                start=True,
                stop=True,
            )
        # dot per edge: dot1[e] = 21 + sum_f aal[e,f] * ghat[e,f]
        prod = dvep.tile([P, HB, d], F32, tag="prod")
        nc.vector.tensor_tensor(
            out=prod[:], in0=aal[:], in1=g[:, :, d : 2 * d],
            op=mybir.AluOpType.mult,
        )
        dot1 = dvep.tile([P, HB], F32, tag="dot1")
        nc.vector.tensor_reduce(
            out=dot1[:], in_=prod[:], op=mybir.AluOpType.add,
            axis=mybir.AxisListType.X,
        )
        nc.vector.tensor_scalar_add(dot1[:], dot1[:], 21.0)
        # wv = dot1 * (0.05*val)  (== val * (1.05 + 0.05*sim_dot))
        wv = dvep.tile([P, HB], F32, tag="wv")
        nc.vector.tensor_tensor(
            out=wv[:], in0=dot1[:], in1=valp_t[:, t0 : t0 + HB],
            op=mybir.AluOpType.mult,
        )
        # weighted messages: msgs[e, sl, f] = wv[e, sl] * g_raw[e, sl, f]
        msgs = msgp.tile([P, HB, d], BF16, tag="msgs")
        nc.vector.tensor_tensor(
            out=msgs[:], in0=g[:, :, 0:d],
            in1=wv[:].to_broadcast([P, HB, d]),
            op=mybir.AluOpType.mult,
        )
        if pend is not None:
            emit_scatter(*pend)
        pend = (c, msgs)
    emit_scatter(*pend)
    # windows with no high-pass tiles: close from acc only
    for w in range(nw):
        if sched.T[1, w] == 0:
            out_close(w, None, acc[:, w * d : (w + 1) * d], sched.T[0, w] > 0)


def build_kernel(cfg: Config, sched1: StageSched, sched2: StageSched):
    nc = bacc.Bacc(
        "TRN2",
        target_bir_lowering=False,
        debug=False,
        enable_asserts=False,
        num_devices=cfg.n_cores,
        num_swdge_queues=4,
    )
    d = cfg.d
    embs = nc.dram_tensor("embs", [cfg.n_nodes, d], F32, kind="ExternalInput")
    sio = {}
    for s, sch in (("s1", sched1), ("s2", sched2)):
        nt = sch.total_tiles
        sio[s] = {
            "idx": nc.dram_tensor(f"{s}_idx", [P, (nt // TB) * (TB * P // 16)], I16, kind="ExternalInput"),
            "valp": nc.dram_tensor(f"{s}_valp", [P, nt], F32, kind="ExternalInput"),
            "spr": nc.dram_tensor(f"{s}_spr", [P, nt * P], F8, kind="ExternalInput"),
            "s8": nc.dram_tensor(f"{s}_s8", [P, nt * P], F8, kind="ExternalInput"),
        }
    out = nc.dram_tensor("out", [cfg.rpc, d], F32, kind="ExternalOutput")
    aug1 = nc.dram_tensor("aug1", [cfg.n_pad, 2 * d], BF16)
    msgtar_local = nc.dram_tensor("mt_loc", [cfg.rpc, d], BF16)
    msgtar_full = nc.dram_tensor("mt_full", [cfg.n_pad, d], BF16)

    with tile.TileContext(nc) as tc:
        cpool = tc.alloc_tile_pool(name="const", bufs=1)
        # constants
        strip = cpool.tile([P, cfg.nw * d], F8)

        # ---------------- prologue: build aug1 + hat strip ----------------
        nblk = cfg.nblk
        CB = 56 if nblk % 56 == 0 else max(
            c for c in range(1, 65) if nblk % c == 0
        )
        nchunks = nblk // CB
        with tc.tile_pool(name="pro", bufs=2) as pro, tc.tile_pool(
            name="pro1", bufs=1
        ) as pro1:
            hatf8 = pro1.tile([P, nblk * d], F8)
            for c in range(nchunks):
                r0 = c * CB * P  # first natural row of chunk
                rows = CB * P
                raw = pro.tile([P, CB, d], F32, tag="praw")
                # rows r0 + t*128 + p; valid rows < n_nodes
                valid = min(rows, max(0, cfg.n_nodes - r0))
                full_t = valid // P
                if valid < rows:
                    nc.vector.memset(raw[:], 0.0)
                if full_t > 0:
                    nc.sync.dma_start(
                        out=raw[:, 0:full_t, :],
                        in_=embs[r0 : r0 + full_t * P].rearrange(
                            "(t p) f -> p t f", p=P
                        ),
                    )
                rem = valid - full_t * P
                if rem > 0:
                    nc.sync.dma_start(
                        out=raw[0:rem, full_t, :],
                        in_=embs[r0 + full_t * P : r0 + valid].rearrange(
                            "(t p) f -> p t f", p=rem
                        )[0:rem, 0, :],
                    )
                tmp = pro.tile([P, CB, d], F32, tag="ptmp")
                nc.vector.tensor_tensor(
                    out=tmp[:], in0=raw[:], in1=raw[:], op=mybir.AluOpType.mult
                )
                nrm2 = pro.tile([P, CB], F32, tag="pnrm")
                nc.vector.tensor_reduce(
                    out=nrm2[:], in_=tmp[:], op=mybir.AluOpType.add,
                    axis=mybir.AxisListType.X,
                )
                nc.vector.tensor_scalar_add(nrm2[:], nrm2[:], 1e-12)
                rec = pro.tile([P, CB], F32, tag="prec")
                nc.vector.reciprocal(rec[:], nrm2[:])
                invn = pro.tile([P, CB], F32, tag="pinv")
                nc.scalar.sqrt(invn[:], rec[:])
                comb = pro.tile([P, CB, 2 * d], BF16, tag="pcomb")
                nc.vector.tensor_copy(out=comb[:, :, 0:d], in_=raw[:])
                nc.vector.tensor_tensor(
                    out=comb[:, :, d : 2 * d], in0=raw[:],
                    in1=invn[:].to_broadcast([P, CB, d]), op=mybir.AluOpType.mult,
                )
                nc.vector.tensor_copy(
                    out=hatf8[:, c * CB * d : (c + 1) * CB * d],
                    in_=comb[:, :, d : 2 * d],
                )
                nc.sync.dma_start(
                    out=aug1[r0 : r0 + rows].rearrange("(t p) f -> p t f", p=P),
                    in_=comb[:],
                )
            # per-core window strip: global window W = pid*nw + w
            pid = nc.vector.partition_id()
            nc.vector.tensor_copy(
                out=strip[:],
                in_=hatf8[:, bass.ds(pid * (cfg.nw * d), cfg.nw * d)],
            )

        # ---------------- stage pools ----------------
        pools = {
            "g": tc.alloc_tile_pool(name="g", bufs=12),
            "aal": tc.alloc_tile_pool(name="aal", bufs=4, space="PSUM"),
            "pacc": tc.alloc_tile_pool(name="pacc", bufs=3, space="PSUM"),
            "spr": tc.alloc_tile_pool(name="sprp", bufs=3),
            "s8": tc.alloc_tile_pool(name="s8p", bufs=3),
            "dve": tc.alloc_tile_pool(name="dve", bufs=8),
            "msg": tc.alloc_tile_pool(name="msg", bufs=6),
            "acc": tc.alloc_tile_pool(name="accp", bufs=1),
            "io": tc.alloc_tile_pool(name="iop", bufs=1),
            "wout": tc.alloc_tile_pool(name="wout", bufs=4),
        }
        consts = {"strip": strip}

        def load_stage_meta(s):
            io = pools["io"]
            t = sio[s]
            nt = t["valp"].shape[1]
            idx_t = io.tile([P, t["idx"].shape[1]], I16, tag=f"{s}i", name=f"{s}i")
            nc.sync.dma_start(idx_t[:], t["idx"][:, :])
            valp_t = io.tile([P, nt], F32, tag=f"{s}v", name=f"{s}v")
            nc.sync.dma_start(valp_t[:], t["valp"][:, :])
            return idx_t, valp_t, t["spr"], t["s8"]

        # ---------------- stage 1 (tar) ----------------
        idx1, valp1, spr1, s81 = load_stage_meta("s1")

        def close1(w, psum_ap, acc_ap, has_low):
            ot = pools["wout"].tile([P, d], BF16, tag="wo1")
            if psum_ap is None:
                nc.scalar.copy(out=ot[:], in_=acc_ap)
            elif has_low:
                nc.vector.tensor_tensor(
                    out=ot[:], in0=acc_ap, in1=psum_ap, op=mybir.AluOpType.add
                )
            else:
                nc.scalar.copy(out=ot[:], in_=psum_ap)
            nc.sync.dma_start(
                out=msgtar_local[w * P : (w + 1) * P, :], in_=ot[:]
            )

        _emit_stage(
            tc, cfg, sched1, pools, consts, aug1[:], idx1, valp1, spr1, s81,
            close1,
        )

        # ---------------- allgather (raw msg_tar only) ----------------
        nc.gpsimd.collective_compute(
            "AllGather",
            mybir.AluOpType.bypass,
            replica_groups=[list(range(cfg.n_cores))],
            ins=[msgtar_local[:].opt()],
            outs=[msgtar_full[:].opt()],
        )
        # merge gathered raw into aug1's raw half (hat half already valid)
        nc.sync.dma_start(out=aug1[:, 0:d], in_=msgtar_full[:, :])

        # ---------------- stage 2 (src) ----------------
        idx2, valp2, spr2, s82 = load_stage_meta("s2")

        def close2(w, psum_ap, acc_ap, has_low):
            ot = pools["wout"].tile([P, d], F32, tag="wo2")
            if psum_ap is None:
                nc.scalar.copy(out=ot[:], in_=acc_ap)
            elif has_low:
                nc.vector.tensor_tensor(
                    out=ot[:], in0=acc_ap, in1=psum_ap, op=mybir.AluOpType.add
                )
            else:
                nc.scalar.copy(out=ot[:], in_=psum_ap)
            nc.sync.dma_start(out=out[w * P : (w + 1) * P, :], in_=ot[:])

        _emit_stage(
            tc, cfg, sched2, pools, consts, aug1[:], idx2, valp2, spr2, s82,
            close2,
        )

        for p in reversed(list(pools.values())):
            p.release()
        cpool.release()

    nc.compile()
    return nc


def prepare(cfg: Config, inputs):
    """inputs: dict with pois_embs, src_edge_index, src_edge_val, tar_*."""
    sched1, meta1 = route_edges(cfg, inputs["tar_edge_index"], inputs["tar_edge_val"])
    sched2, meta2 = route_edges(cfg, inputs["src_edge_index"], inputs["src_edge_val"])
    embs = np.asarray(inputs["pois_embs"], dtype=np.float32)
    in_maps = []
    for k in range(cfg.n_cores):
        in_maps.append(
            {
                "embs": embs,
                "s1_idx": meta1[k]["idx"], "s1_valp": meta1[k]["valp"],
                "s1_spr": meta1[k]["spr"], "s1_s8": meta1[k]["s8"],
                "s2_idx": meta2[k]["idx"], "s2_valp": meta2[k]["valp"],
                "s2_spr": meta2[k]["spr"], "s2_s8": meta2[k]["s8"],
            }
        )
    return sched1, sched2, in_maps


def assemble_output(cfg: Config, results):
    out = np.zeros((cfg.n_nodes, cfg.d), dtype=np.float32)
    for k, r in enumerate(results):
        lo = k * cfg.rpc
        hi = min(lo + cfg.rpc, cfg.n_nodes)
        if hi > lo:
            out[lo:hi] = r["out"][0 : hi - lo]
    return out


_CACHE = {}


def kernel(**inputs):
    import concourse.bass_utils as bass_utils

    cfg = Config()
    sched1, sched2, in_maps = prepare(cfg, inputs)
    key = (sched1.n_tiles, sched2.n_tiles, tuple(sched1.T.ravel()), tuple(sched2.T.ravel()))
    nc = _CACHE.get(key)
    if nc is None:
        nc = build_kernel(cfg, sched1, sched2)
        _CACHE[key] = nc
    res = bass_utils.run_bass_kernel_spmd(
        nc, in_maps, core_ids=list(range(cfg.n_cores)), trace=False
    )
    out = assemble_output(cfg, res.results)
    return out.astype(np.float32, copy=False)


# revision 8
# speedup vs baseline: 2.1405x; 1.1358x over previous
"""Trainium2 Bass kernel for nn_DirectedHyperConvLayer (GNN message passing).

Self-contained: accepts FULL inputs, shards across 8 NeuronCores internally,
returns the FULL [50000, 64] float32 output.

Sharding: each core owns a contiguous block of destination rows; the host
routes/sorts edges by destination row, pads them into 128-edge tiles grouped
by 128-row destination windows, and splits each window's edges into low/high
passes so source indices fit dma_gather's int16 limit. On device, combined
[raw_bf16 | hat_bf16] rows are fetched with batched dma_gather calls; the
per-edge cosine dot hat_dst . hat_src runs on the vector engine against a
host-expanded destination-hat stream; the weight wv = (21 + dot) * (0.05*val)
scales raw gathered rows into messages, which a host-supplied f8 one-hot
(s8, [edge, dest-row]) segment-sums into PSUM per window through the tensor
engine. Stage 1 results (raw bf16) are AllGathered and merged into the
gather table's raw half for stage 2. Scatter matmuls for call c are emitted
during call c+1 so the tensor engine's in-order queue never stalls behind
the dot/weight chain of the current call.
"""


from dataclasses import dataclass

import numpy as np
import ml_dtypes

import concourse.bass as bass
import concourse.bacc as bacc
import concourse.mybir as mybir
import concourse.tile as tile

F32 = mybir.dt.float32
BF16 = mybir.dt.bfloat16
F8 = mybir.dt.float8e4
I16 = mybir.dt.int16
NP_F8 = mybir.dt.np(F8)
NP_BF16 = ml_dtypes.bfloat16
P = 128
TB = 8  # tiles per gather call / compute batch (1024 idx)


@dataclass
class Config:
    n_nodes: int = 50000
    d: int = 64
    n_cores: int = 8
    rpc: int = 6272  # rows per core (multiple of 128)
    split: int = 32768  # low/high gather split (<= 32768)

    @property
    def nw(self):
        return self.rpc // P

    @property
    def n_pad(self):
        return self.n_cores * self.rpc


@dataclass
class StageSched:
    T: np.ndarray  # [2, nw] tiles per (pass, window)
    n_tiles: tuple  # (low, high) tile counts (each % TB == 0)

    @property
    def total_tiles(self):
        return int(self.n_tiles[0] + self.n_tiles[1])

    def tile_windows(self):
        """list over global tile index -> (pass, w, j_in_window, first, last)"""
        out = []
        for p in range(2):
            for w in range(self.T.shape[1]):
                Tw = int(self.T[p, w])
                for j in range(Tw):
                    out.append((p, w, j, j == 0, j == Tw - 1))
        return out


def make_hat(embs):
    """[N, d] f32 -> (raw bf16, hat bf16)."""
    nrm = np.sqrt((embs.astype(np.float64) ** 2).sum(-1))
    nrm = np.maximum(nrm, 1e-8)
    hat = (embs / nrm[:, None].astype(np.float32)).astype(NP_BF16)
    return embs.astype(NP_BF16), hat


def route_edges(cfg: Config, edge_index, edge_val, hat):
    """Returns (sched, per_core list of dicts with idx/valp/s8/hd)."""
    r0 = np.asarray(edge_index[0], dtype=np.int64)
    r1 = np.asarray(edge_index[1], dtype=np.int64)
    val = np.asarray(edge_val, dtype=np.float32)
    E = r0.shape[0]
    nc_, nw = cfg.n_cores, cfg.nw

    k = r0 // cfg.rpc
    w = (r0 % cfg.rpc) // P
    dloc = r0 % P
    hi = (r1 >= cfg.split).astype(np.int64)
    gid = (k * 2 + hi) * nw + w

    counts = np.bincount(gid, minlength=nc_ * 2 * nw).reshape(nc_, 2, nw)
    T = np.ceil(counts.max(axis=0) / P).astype(np.int64)  # [2, nw]
    # pad each pass's tile total to a multiple of TB
    for pss in range(2):
        T[pss, nw - 1] += (-int(T[pss].sum())) % TB
    nt_low, nt_high = int(T[0].sum()), int(T[1].sum())
    n_tiles = nt_low + nt_high
    # global tile base per (pass, w)
    tbase = np.zeros((2, nw), dtype=np.int64)
    tbase[0] = np.cumsum(T[0]) - T[0]
    tbase[1] = nt_low + np.cumsum(T[1]) - T[1]

    # slot within (k, hi, w) group
    order = np.argsort(gid, kind="stable")
    sorted_gid = gid[order]
    starts = np.searchsorted(sorted_gid, np.arange(nc_ * 2 * nw))
    ranks = np.empty(E, dtype=np.int64)
    ranks[order] = np.arange(E) - starts[sorted_gid]

    tile_g = tbase[hi, w] + ranks // P  # global tile per edge
    pos = ranks % P
    idx_val = (r1 - hi * cfg.split).astype(np.int16)

    ncalls = n_tiles // TB
    cw = TB * P // 16
    per_core = []
    for kk in range(nc_):
        m = k == kk
        tg, pg = tile_g[m], pos[m]
        slots = tg * P + pg
        idx_flat = np.zeros(n_tiles * P, dtype=np.int16)
        valp_flat = np.zeros(n_tiles * P, dtype=np.float32)
        idx_flat[slots] = idx_val[m]
        valp_flat[slots] = 0.05 * val[m]
        # idx16 wrapped: [128, ncalls*(TB*P//16)]
        iw = idx_flat.reshape(ncalls, cw, 16)
        iw = np.transpose(iw, (2, 0, 1)).reshape(16, ncalls * cw)
        idx16 = np.tile(iw, (8, 1))
        # valp [128, n_tiles] (partition p, tile t)
        valp2d = valp_flat.reshape(n_tiles, P).T.copy()
        # s8 [128(e), n_tiles*128(d)] f8 one-hot
        dl = dloc[m]
        s8 = np.zeros((P, n_tiles * P), dtype=NP_F8)
        s8[pg, tg * P + dl] = NP_F8(1.0)
        # hd [128(e), n_tiles*64] bf16: dest-row hat per edge slot
        hd = np.zeros((P, n_tiles, cfg.d), dtype=NP_BF16)
        hd[pg, tg] = hat[r0[m]]
        per_core.append(
            {
                "idx": idx16,
                "valp": valp2d,
                "s8": s8,
                "hd": hd.reshape(P, n_tiles * cfg.d),
            }
        )
    return StageSched(T=T, n_tiles=(nt_low, nt_high)), per_core


def _emit_stage(
    tc, cfg, sched: StageSched, pools, table, idx_t, valp_t, s8_dram, hd_dram,
    out_close,
):
    """Emit one spmm stage. out_close(w, psum_ap, acc_ap, has_low) writes the
    finished window."""
    nc = tc.nc
    nw = cfg.nw
    d = cfg.d
    gp, paccp, dvep, msgp, accp, s8p, hdp = (
        pools["g"], pools["pacc"], pools["dve"], pools["msg"], pools["acc"],
        pools["s8"], pools["hd"],
    )

    acc = accp.tile([P, nw * d], F32, tag="acc")
    nc.vector.memset(acc[:], 0.0)

    tw = sched.tile_windows()
    n_tiles = sched.total_tiles
    assert n_tiles % TB == 0
    ncalls = n_tiles // TB
    nt_low = sched.n_tiles[0]
    cw = TB * P // 16  # idx cols per call

    # s8 one-hot tiles, streamed per (pass, w)
    s8_tiles = {}
    st0 = 0
    for pss in range(2):
        for w in range(nw):
            Tw = int(sched.T[pss, w])
            if Tw == 0:
                continue
            s8t = s8p.tile([P, Tw * P], F8, tag="s8", name="s8")
            nc.sync.dma_start(s8t[:], s8_dram[:, st0 * P : (st0 + Tw) * P])
            s8_tiles[(pss, w)] = (s8t, st0)
            st0 += Tw

    win_psum = {}

    def emit_scatter(c, msgs):
        t0 = c * TB
        for sl in range(TB):
            t = t0 + sl
            pss_t, w, j, first, last = tw[t]
            key = (pss_t, w)
            if key not in win_psum:
                win_psum[key] = paccp.tile(
                    [P, d], F32, space="PSUM", tag="pacc", name="pacc"
                )
            pw = win_psum[key]
            s8t, st_t0 = s8_tiles[key]
            jj = t - st_t0
            nc.tensor.matmul(
                out=pw[:],
                lhsT=s8t[:, jj * P : (jj + 1) * P],
                rhs=msgs[:, sl, :],
                start=first,
                stop=last,
            )
            if last:
                if pss_t == 0:
                    nc.scalar.copy(out=acc[:, w * d : (w + 1) * d], in_=pw[:])
                else:
                    has_low = sched.T[0, w] > 0
                    out_close(w, pw[:], acc[:, w * d : (w + 1) * d], has_low)
                del win_psum[key]

    pend = None  # (c, msgs) awaiting scatter
    for c in range(ncalls):
        pss = 0 if c * TB < nt_low else 1
        tab = table[0 : cfg.split, :] if pss == 0 else table[cfg.split : cfg.n_pad, :]
        g = gp.tile([P, TB, 2 * d], BF16, tag="g")
        nc.gpsimd.dma_gather(
            out_ap=g[:],
            in_ap=tab,
            idxs_ap=idx_t[:, c * cw : (c + 1) * cw],
            num_idxs=TB * P,
            num_idxs_reg=TB * P,
            elem_size=2 * d,
            queue_num=c % 4,
            single_packet=True,
        )
        t0 = c * TB
        hd = hdp.tile([P, TB, d], BF16, tag="hd")
        nc.sync.dma_start(hd[:], hd_dram[:, t0 * d : (t0 + TB) * d])
        # dot per edge: dot1[e] = 21 + sum_f hd[e,f] * ghat[e,f]
        prod = dvep.tile([P, TB, d], F32, tag="prod")
        nc.vector.tensor_tensor(
            out=prod[:], in0=hd[:], in1=g[:, :, d : 2 * d],
            op=mybir.AluOpType.mult,
        )
        dot1 = dvep.tile([P, TB], F32, tag="dot1")
        nc.vector.tensor_reduce(
            out=dot1[:], in_=prod[:], op=mybir.AluOpType.add,
            axis=mybir.AxisListType.X,
        )
        nc.vector.tensor_scalar_add(dot1[:], dot1[:], 21.0)
        # wv = dot1 * (0.05*val)  (== val * (1.05 + 0.05*sim_dot))
        wv = dvep.tile([P, TB], F32, tag="wv")
        nc.vector.tensor_tensor(
            out=wv[:], in0=dot1[:], in1=valp_t[:, t0 : t0 + TB],
            op=mybir.AluOpType.mult,
        )
        # weighted messages: msgs[e, sl, f] = wv[e, sl] * g_raw[e, sl, f]
        msgs = msgp.tile([P, TB, d], BF16, tag="msgs")
        nc.vector.tensor_tensor(
            out=msgs[:], in0=g[:, :, 0:d],
            in1=wv[:].to_broadcast([P, TB, d]),
            op=mybir.AluOpType.mult,
        )
        if pend is not None:
            emit_scatter(*pend)
        pend = (c, msgs)
    emit_scatter(*pend)
    # windows with no high-pass tiles: close from acc only
    for w in range(nw):
        if sched.T[1, w] == 0:
            out_close(w, None, acc[:, w * d : (w + 1) * d], sched.T[0, w] > 0)


def build_kernel(cfg: Config, sched1: StageSched, sched2: StageSched):
    nc = bacc.Bacc(
        "TRN2",
        target_bir_lowering=False,
        debug=False,
        enable_asserts=False,
        num_devices=cfg.n_cores,
        num_swdge_queues=4,
    )
    d = cfg.d
    aug1 = nc.dram_tensor("aug1", [cfg.n_pad, 2 * d], BF16, kind="ExternalInput")
    sio = {}
    for s, sch in (("s1", sched1), ("s2", sched2)):
        nt = sch.total_tiles
        sio[s] = {
            "idx": nc.dram_tensor(
                f"{s}_idx", [P, (nt // TB) * (TB * P // 16)], I16,
                kind="ExternalInput",
            ),
            "valp": nc.dram_tensor(f"{s}_valp", [P, nt], F32, kind="ExternalInput"),
            "s8": nc.dram_tensor(f"{s}_s8", [P, nt * P], F8, kind="ExternalInput"),
            "hd": nc.dram_tensor(f"{s}_hd", [P, nt * d], BF16, kind="ExternalInput"),
        }
    out = nc.dram_tensor("out", [cfg.rpc, d], F32, kind="ExternalOutput")
    aug2 = nc.dram_tensor("aug2", [cfg.n_pad, 2 * d], BF16)
    msgtar_local = nc.dram_tensor("mt_loc", [cfg.rpc, d], BF16)
    msgtar_full = nc.dram_tensor("mt_full", [cfg.n_pad, d], BF16)

    with tile.TileContext(nc) as tc:
        pools = {
            "g": tc.alloc_tile_pool(name="g", bufs=10),
            "pacc": tc.alloc_tile_pool(name="pacc", bufs=6, space="PSUM"),
            "s8": tc.alloc_tile_pool(name="s8p", bufs=3),
            "hd": tc.alloc_tile_pool(name="hdp", bufs=8),
            "dve": tc.alloc_tile_pool(name="dve", bufs=6),
            "msg": tc.alloc_tile_pool(name="msg", bufs=6),
            "acc": tc.alloc_tile_pool(name="accp", bufs=1),
            "io": tc.alloc_tile_pool(name="iop", bufs=1),
            "wout": tc.alloc_tile_pool(name="wout", bufs=4),
        }

        def load_stage_meta(s):
            io = pools["io"]
            t = sio[s]
            nt = t["valp"].shape[1]
            idx_t = io.tile([P, t["idx"].shape[1]], I16, tag=f"{s}i", name=f"{s}i")
            nc.sync.dma_start(idx_t[:], t["idx"][:, :])
            valp_t = io.tile([P, nt], F32, tag=f"{s}v", name=f"{s}v")
            nc.sync.dma_start(valp_t[:], t["valp"][:, :])
            return idx_t, valp_t, t["s8"], t["hd"]

        # stage-2 table hat half: copy early, overlaps stage 1
        nc.sync.dma_start(out=aug2[:, d : 2 * d], in_=aug1[:, d : 2 * d])

        # ---------------- stage 1 (tar) ----------------
        idx1, valp1, s81, hd1 = load_stage_meta("s1")

        def close1(w, psum_ap, acc_ap, has_low):
            ot = pools["wout"].tile([P, d], BF16, tag="wo1")
            if psum_ap is None:
                nc.scalar.copy(out=ot[:], in_=acc_ap)
            elif has_low:
                nc.vector.tensor_tensor(
                    out=ot[:], in0=acc_ap, in1=psum_ap, op=mybir.AluOpType.add
                )
            else:
                nc.scalar.copy(out=ot[:], in_=psum_ap)
            nc.sync.dma_start(
                out=msgtar_local[w * P : (w + 1) * P, :], in_=ot[:]
            )

        _emit_stage(
            tc, cfg, sched1, pools, aug1[:], idx1, valp1, s81[:, :], hd1[:, :],
            close1,
        )

        # ---------------- allgather (raw msg_tar only) ----------------
        nc.gpsimd.collective_compute(
            "AllGather",
            mybir.AluOpType.bypass,
            replica_groups=[list(range(cfg.n_cores))],
            ins=[msgtar_local[:].opt()],
            outs=[msgtar_full[:].opt()],
        )
        # merge gathered raw into aug2's raw half (hat half copied earlier)
        nc.sync.dma_start(out=aug2[:, 0:d], in_=msgtar_full[:, :])

        # ---------------- stage 2 (src) ----------------
        idx2, valp2, s82, hd2 = load_stage_meta("s2")

        def close2(w, psum_ap, acc_ap, has_low):
            ot = pools["wout"].tile([P, d], F32, tag="wo2")
            if psum_ap is None:
                nc.scalar.copy(out=ot[:], in_=acc_ap)
            elif has_low:
                nc.vector.tensor_tensor(
                    out=ot[:], in0=acc_ap, in1=psum_ap, op=mybir.AluOpType.add
                )
            else:
                nc.scalar.copy(out=ot[:], in_=psum_ap)
            nc.sync.dma_start(out=out[w * P : (w + 1) * P, :], in_=ot[:])

        _emit_stage(
            tc, cfg, sched2, pools, aug2[:], idx2, valp2, s82[:, :], hd2[:, :],
            close2,
        )

        for p in reversed(list(pools.values())):
            p.release()

    nc.compile()
    return nc


def prepare(cfg: Config, inputs):
    """inputs: dict with pois_embs, src_edge_index, src_edge_val, tar_*."""
    embs = np.asarray(inputs["pois_embs"], dtype=np.float32)
    raw, hat = make_hat(embs)
    aug = np.zeros((cfg.n_pad, 2 * cfg.d), dtype=NP_BF16)
    aug[: cfg.n_nodes, : cfg.d] = raw
    aug[: cfg.n_nodes, cfg.d :] = hat
    sched1, meta1 = route_edges(
        cfg, inputs["tar_edge_index"], inputs["tar_edge_val"], hat
    )
    sched2, meta2 = route_edges(
        cfg, inputs["src_edge_index"], inputs["src_edge_val"], hat
    )
    in_maps = []
    for k in range(cfg.n_cores):
        in_maps.append(
            {
                "aug1": aug,
                "s1_idx": meta1[k]["idx"], "s1_valp": meta1[k]["valp"],
                "s1_s8": meta1[k]["s8"], "s1_hd": meta1[k]["hd"],
                "s2_idx": meta2[k]["idx"], "s2_valp": meta2[k]["valp"],
                "s2_s8": meta2[k]["s8"], "s2_hd": meta2[k]["hd"],
            }
        )
    return sched1, sched2, in_maps


def assemble_output(cfg: Config, results):
    out = np.zeros((cfg.n_nodes, cfg.d), dtype=np.float32)
    for k, r in enumerate(results):
        lo = k * cfg.rpc
        hi = min(lo + cfg.rpc, cfg.n_nodes)
        if hi > lo:
            out[lo:hi] = r["out"][0 : hi - lo]
    return out


_CACHE = {}


def kernel(**inputs):
    import concourse.bass_utils as bass_utils

    cfg = Config()
    sched1, sched2, in_maps = prepare(cfg, inputs)
    key = (
        sched1.n_tiles, sched2.n_tiles,
        tuple(sched1.T.ravel()), tuple(sched2.T.ravel()),
    )
    nc = _CACHE.get(key)
    if nc is None:
        nc = build_kernel(cfg, sched1, sched2)
        _CACHE[key] = nc
    res = bass_utils.run_bass_kernel_spmd(
        nc, in_maps, core_ids=list(range(cfg.n_cores)), trace=False
    )
    out = assemble_output(cfg, res.results)
    return out.astype(np.float32, copy=False)


# revision 9
# speedup vs baseline: 2.1800x; 1.0185x over previous
"""Trainium2 Bass kernel for nn_DirectedHyperConvLayer (GNN message passing).

Self-contained: accepts FULL inputs, shards across 8 NeuronCores internally,
returns the FULL [50000, 64] float32 output.

Sharding: each core owns a contiguous block of destination rows; the host
routes/sorts edges by destination row, pads them into 128-edge tiles grouped
by 128-row destination windows, and splits each window's edges into low/high
passes so source indices fit dma_gather's int16 limit. On device, combined
[raw_bf16 | hat_bf16] rows are fetched with batched dma_gather calls; the
per-edge cosine dot hat_dst . hat_src runs on the vector engine against a
host-expanded destination-hat stream; the weight wv = (21 + dot) * (0.05*val)
scales raw gathered rows into messages, which a host-supplied f8 one-hot
(s8, [edge, dest-row]) segment-sums into PSUM per window through the tensor
engine. Stage 1 results (raw bf16) are AllGathered and merged into the
gather table's raw half for stage 2. Scatter matmuls for call c are emitted
during call c+1 so the tensor engine's in-order queue never stalls behind
the dot/weight chain of the current call.
"""


from dataclasses import dataclass

import numpy as np
import ml_dtypes

import concourse.bass as bass
import concourse.bacc as bacc
import concourse.mybir as mybir
import concourse.tile as tile

F32 = mybir.dt.float32
BF16 = mybir.dt.bfloat16
F8 = mybir.dt.float8e4
I16 = mybir.dt.int16
NP_F8 = mybir.dt.np(F8)
NP_BF16 = ml_dtypes.bfloat16
P = 128
TB = 8  # tiles per gather call / compute batch (1024 idx)


@dataclass
class Config:
    n_nodes: int = 50000
    d: int = 64
    n_cores: int = 8
    rpc: int = 6272  # rows per core (multiple of 128)
    split: int = 32768  # low/high gather split (<= 32768)

    @property
    def nw(self):
        return self.rpc // P

    @property
    def n_pad(self):
        return self.n_cores * self.rpc


@dataclass
class StageSched:
    T: np.ndarray  # [2, nw] tiles per (pass, window)
    n_tiles: tuple  # (low, high) tile counts (each % TB == 0)

    @property
    def total_tiles(self):
        return int(self.n_tiles[0] + self.n_tiles[1])

    def tile_windows(self):
        """list over global tile index -> (pass, w, j_in_window, first, last)"""
        out = []
        for p in range(2):
            for w in range(self.T.shape[1]):
                Tw = int(self.T[p, w])
                for j in range(Tw):
                    out.append((p, w, j, j == 0, j == Tw - 1))
        return out


def make_hat(embs):
    """[N, d] f32 -> (raw bf16, hat bf16)."""
    nrm = np.sqrt((embs.astype(np.float64) ** 2).sum(-1))
    nrm = np.maximum(nrm, 1e-8)
    hat = (embs / nrm[:, None].astype(np.float32)).astype(NP_BF16)
    return embs.astype(NP_BF16), hat


def route_edges(cfg: Config, edge_index, edge_val, hat):
    """Returns (sched, per_core list of dicts with idx/valp/s8/hd)."""
    r0 = np.asarray(edge_index[0], dtype=np.int64)
    r1 = np.asarray(edge_index[1], dtype=np.int64)
    val = np.asarray(edge_val, dtype=np.float32)
    E = r0.shape[0]
    nc_, nw = cfg.n_cores, cfg.nw

    k = r0 // cfg.rpc
    w = (r0 % cfg.rpc) // P
    dloc = r0 % P
    hi = (r1 >= cfg.split).astype(np.int64)
    gid = (k * 2 + hi) * nw + w

    counts = np.bincount(gid, minlength=nc_ * 2 * nw).reshape(nc_, 2, nw)
    T = np.ceil(counts.max(axis=0) / P).astype(np.int64)  # [2, nw]
    # pad each pass's tile total to a multiple of TB
    for pss in range(2):
        T[pss, nw - 1] += (-int(T[pss].sum())) % TB
    nt_low, nt_high = int(T[0].sum()), int(T[1].sum())
    n_tiles = nt_low + nt_high
    # global tile base per (pass, w)
    tbase = np.zeros((2, nw), dtype=np.int64)
    tbase[0] = np.cumsum(T[0]) - T[0]
    tbase[1] = nt_low + np.cumsum(T[1]) - T[1]

    # slot within (k, hi, w) group
    order = np.argsort(gid, kind="stable")
    sorted_gid = gid[order]
    starts = np.searchsorted(sorted_gid, np.arange(nc_ * 2 * nw))
    ranks = np.empty(E, dtype=np.int64)
    ranks[order] = np.arange(E) - starts[sorted_gid]

    tile_g = tbase[hi, w] + ranks // P  # global tile per edge
    pos = ranks % P
    idx_val = (r1 - hi * cfg.split).astype(np.int16)

    ncalls = n_tiles // TB
    cw = TB * P // 16
    per_core = []
    for kk in range(nc_):
        m = k == kk
        tg, pg = tile_g[m], pos[m]
        slots = tg * P + pg
        idx_flat = np.zeros(n_tiles * P, dtype=np.int16)
        valp_flat = np.zeros(n_tiles * P, dtype=np.float32)
        idx_flat[slots] = idx_val[m]
        valp_flat[slots] = 0.05 * val[m]
        # idx16 wrapped: [128, ncalls*(TB*P//16)]
        iw = idx_flat.reshape(ncalls, cw, 16)
        iw = np.transpose(iw, (2, 0, 1)).reshape(16, ncalls * cw)
        idx16 = np.tile(iw, (8, 1))
        # valp [128, n_tiles] (partition p, tile t)
        valp2d = valp_flat.reshape(n_tiles, P).T.copy()
        # s8 [128(e), n_tiles*128(d)] f8 one-hot
        dl = dloc[m]
        s8 = np.zeros((P, n_tiles * P), dtype=NP_F8)
        s8[pg, tg * P + dl] = NP_F8(1.0)
        # hd [128(e), n_tiles*64] bf16: dest-row hat per edge slot
        hd = np.zeros((P, n_tiles, cfg.d), dtype=NP_BF16)
        hd[pg, tg] = hat[r0[m]]
        per_core.append(
            {
                "idx": idx16,
                "valp": valp2d,
                "s8": s8,
                "hd": hd.reshape(P, n_tiles * cfg.d),
            }
        )
    return StageSched(T=T, n_tiles=(nt_low, nt_high)), per_core


def _emit_stage(
    tc, cfg, sched: StageSched, pools, table, idx_t, valp_t, s8_dram, hd_dram,
    out_close,
):
    """Emit one spmm stage. out_close(w, psum_ap, acc_ap, has_low) writes the
    finished window."""
    nc = tc.nc
    nw = cfg.nw
    d = cfg.d
    gp, paccp, dvep, msgp, accp, s8p, hdp = (
        pools["g"], pools["pacc"], pools["dve"], pools["msg"], pools["acc"],
        pools["s8"], pools["hd"],
    )

    acc = accp.tile([P, nw * d], F32, tag="acc")
    nc.vector.memset(acc[:], 0.0)

    tw = sched.tile_windows()
    n_tiles = sched.total_tiles
    assert n_tiles % TB == 0
    ncalls = n_tiles // TB
    nt_low = sched.n_tiles[0]
    cw = TB * P // 16  # idx cols per call

    # s8 one-hot tiles, streamed per (pass, w)
    s8_tiles = {}
    st0 = 0
    for pss in range(2):
        for w in range(nw):
            Tw = int(sched.T[pss, w])
            if Tw == 0:
                continue
            s8t = s8p.tile([P, Tw * P], F8, tag="s8", name="s8")
            nc.sync.dma_start(s8t[:], s8_dram[:, st0 * P : (st0 + Tw) * P])
            s8_tiles[(pss, w)] = (s8t, st0)
            st0 += Tw

    win_psum = {}

    def emit_scatter(c, msgs):
        t0 = c * TB
        for sl in range(TB):
            t = t0 + sl
            pss_t, w, j, first, last = tw[t]
            key = (pss_t, w)
            if key not in win_psum:
                win_psum[key] = paccp.tile(
                    [P, d], F32, space="PSUM", tag="pacc", name="pacc"
                )
            pw = win_psum[key]
            s8t, st_t0 = s8_tiles[key]
            jj = t - st_t0
            nc.tensor.matmul(
                out=pw[:],
                lhsT=s8t[:, jj * P : (jj + 1) * P],
                rhs=msgs[:, sl, :],
                start=first,
                stop=last,
            )
            if last:
                if pss_t == 0:
                    nc.scalar.copy(out=acc[:, w * d : (w + 1) * d], in_=pw[:])
                else:
                    has_low = sched.T[0, w] > 0
                    out_close(w, pw[:], acc[:, w * d : (w + 1) * d], has_low)
                del win_psum[key]

    pend = None  # (c, msgs) awaiting scatter
    for c in range(ncalls):
        pss = 0 if c * TB < nt_low else 1
        tab = table[0 : cfg.split, :] if pss == 0 else table[cfg.split : cfg.n_pad, :]
        g = gp.tile([P, TB, 2 * d], BF16, tag="g")
        nc.gpsimd.dma_gather(
            out_ap=g[:],
            in_ap=tab,
            idxs_ap=idx_t[:, c * cw : (c + 1) * cw],
            num_idxs=TB * P,
            num_idxs_reg=TB * P,
            elem_size=2 * d,
            queue_num=c % 4,
            single_packet=True,
        )
        t0 = c * TB
        hd = hdp.tile([P, TB, d], BF16, tag="hd")
        nc.sync.dma_start(hd[:], hd_dram[:, t0 * d : (t0 + TB) * d])
        # dot per edge: dot1[e] = 21 + sum_f hd[e,f] * ghat[e,f]
        prod = dvep.tile([P, TB, d], BF16, tag="prod")
        nc.vector.tensor_tensor(
            out=prod[:], in0=hd[:], in1=g[:, :, d : 2 * d],
            op=mybir.AluOpType.mult,
        )
        dot1 = dvep.tile([P, TB], F32, tag="dot1")
        nc.vector.tensor_reduce(
            out=dot1[:], in_=prod[:], op=mybir.AluOpType.add,
            axis=mybir.AxisListType.X,
        )
        nc.vector.tensor_scalar_add(dot1[:], dot1[:], 21.0)
        # wv = dot1 * (0.05*val)  (== val * (1.05 + 0.05*sim_dot))
        wv = dvep.tile([P, TB], F32, tag="wv")
        nc.vector.tensor_tensor(
            out=wv[:], in0=dot1[:], in1=valp_t[:, t0 : t0 + TB],
            op=mybir.AluOpType.mult,
        )
        # weighted messages: msgs[e, sl, f] = wv[e, sl] * g_raw[e, sl, f]
        wvb = dvep.tile([P, TB], BF16, tag="wvb")
        nc.vector.tensor_copy(out=wvb[:], in_=wv[:])
        msgs = msgp.tile([P, TB, d], BF16, tag="msgs")
        nc.vector.tensor_tensor(
            out=msgs[:], in0=g[:, :, 0:d],
            in1=wvb[:].to_broadcast([P, TB, d]),
            op=mybir.AluOpType.mult,
        )
        if pend is not None:
            emit_scatter(*pend)
        pend = (c, msgs)
    emit_scatter(*pend)
    # windows with no high-pass tiles: close from acc only
    for w in range(nw):
        if sched.T[1, w] == 0:
            out_close(w, None, acc[:, w * d : (w + 1) * d], sched.T[0, w] > 0)


def build_kernel(cfg: Config, sched1: StageSched, sched2: StageSched):
    nc = bacc.Bacc(
        "TRN2",
        target_bir_lowering=False,
        debug=False,
        enable_asserts=False,
        num_devices=cfg.n_cores,
        num_swdge_queues=4,
    )
    d = cfg.d
    aug1 = nc.dram_tensor("aug1", [cfg.n_pad, 2 * d], BF16, kind="ExternalInput")
    sio = {}
    for s, sch in (("s1", sched1), ("s2", sched2)):
        nt = sch.total_tiles
        sio[s] = {
            "idx": nc.dram_tensor(
                f"{s}_idx", [P, (nt // TB) * (TB * P // 16)], I16,
                kind="ExternalInput",
            ),
            "valp": nc.dram_tensor(f"{s}_valp", [P, nt], F32, kind="ExternalInput"),
            "s8": nc.dram_tensor(f"{s}_s8", [P, nt * P], F8, kind="ExternalInput"),
            "hd": nc.dram_tensor(f"{s}_hd", [P, nt * d], BF16, kind="ExternalInput"),
        }
    out = nc.dram_tensor("out", [cfg.rpc, d], F32, kind="ExternalOutput")
    aug2 = nc.dram_tensor("aug2", [cfg.n_pad, 2 * d], BF16)
    msgtar_local = nc.dram_tensor("mt_loc", [cfg.rpc, d], BF16)
    msgtar_full = nc.dram_tensor("mt_full", [cfg.n_pad, d], BF16)

    with tile.TileContext(nc) as tc:
        pools = {
            "g": tc.alloc_tile_pool(name="g", bufs=16),
            "pacc": tc.alloc_tile_pool(name="pacc", bufs=6, space="PSUM"),
            "s8": tc.alloc_tile_pool(name="s8p", bufs=3),
            "hd": tc.alloc_tile_pool(name="hdp", bufs=12),
            "dve": tc.alloc_tile_pool(name="dve", bufs=10),
            "msg": tc.alloc_tile_pool(name="msg", bufs=10),
            "acc": tc.alloc_tile_pool(name="accp", bufs=1),
            "io": tc.alloc_tile_pool(name="iop", bufs=1),
            "wout": tc.alloc_tile_pool(name="wout", bufs=4),
        }

        def load_stage_meta(s):
            io = pools["io"]
            t = sio[s]
            nt = t["valp"].shape[1]
            idx_t = io.tile([P, t["idx"].shape[1]], I16, tag=f"{s}i", name=f"{s}i")
            nc.sync.dma_start(idx_t[:], t["idx"][:, :])
            valp_t = io.tile([P, nt], F32, tag=f"{s}v", name=f"{s}v")
            nc.sync.dma_start(valp_t[:], t["valp"][:, :])
            return idx_t, valp_t, t["s8"], t["hd"]

        # ---------------- stage 1 (tar) ----------------
        idx1, valp1, s81, hd1 = load_stage_meta("s1")

        def close1(w, psum_ap, acc_ap, has_low):
            ot = pools["wout"].tile([P, d], BF16, tag="wo1")
            if psum_ap is None:
                nc.scalar.copy(out=ot[:], in_=acc_ap)
            elif has_low:
                nc.vector.tensor_tensor(
                    out=ot[:], in0=acc_ap, in1=psum_ap, op=mybir.AluOpType.add
                )
            else:
                nc.scalar.copy(out=ot[:], in_=psum_ap)
            nc.sync.dma_start(
                out=msgtar_local[w * P : (w + 1) * P, :], in_=ot[:]
            )

        _emit_stage(
            tc, cfg, sched1, pools, aug1[:], idx1, valp1, s81[:, :], hd1[:, :],
            close1,
        )

        # stage-2 table hat half (overlaps stage-1 tail / collective)
        nc.scalar.dma_start(out=aug2[:, d : 2 * d], in_=aug1[:, d : 2 * d])

        # ---------------- allgather (raw msg_tar only) ----------------
        nc.gpsimd.collective_compute(
            "AllGather",
            mybir.AluOpType.bypass,
            replica_groups=[list(range(cfg.n_cores))],
            ins=[msgtar_local[:].opt()],
            outs=[msgtar_full[:].opt()],
        )
        # merge gathered raw into aug2's raw half (hat half copied earlier)
        nc.sync.dma_start(out=aug2[:, 0:d], in_=msgtar_full[:, :])

        # ---------------- stage 2 (src) ----------------
        idx2, valp2, s82, hd2 = load_stage_meta("s2")

        def close2(w, psum_ap, acc_ap, has_low):
            ot = pools["wout"].tile([P, d], F32, tag="wo2")
            if psum_ap is None:
                nc.scalar.copy(out=ot[:], in_=acc_ap)
            elif has_low:
                nc.vector.tensor_tensor(
                    out=ot[:], in0=acc_ap, in1=psum_ap, op=mybir.AluOpType.add
                )
            else:
                nc.scalar.copy(out=ot[:], in_=psum_ap)
            nc.sync.dma_start(out=out[w * P : (w + 1) * P, :], in_=ot[:])

        _emit_stage(
            tc, cfg, sched2, pools, aug2[:], idx2, valp2, s82[:, :], hd2[:, :],
            close2,
        )

        for p in reversed(list(pools.values())):
            p.release()

    nc.compile()
    return nc


def prepare(cfg: Config, inputs):
    """inputs: dict with pois_embs, src_edge_index, src_edge_val, tar_*."""
    embs = np.asarray(inputs["pois_embs"], dtype=np.float32)
    raw, hat = make_hat(embs)
    aug = np.zeros((cfg.n_pad, 2 * cfg.d), dtype=NP_BF16)
    aug[: cfg.n_nodes, : cfg.d] = raw
    aug[: cfg.n_nodes, cfg.d :] = hat
    sched1, meta1 = route_edges(
        cfg, inputs["tar_edge_index"], inputs["tar_edge_val"], hat
    )
    sched2, meta2 = route_edges(
        cfg, inputs["src_edge_index"], inputs["src_edge_val"], hat
    )
    in_maps = []
    for k in range(cfg.n_cores):
        in_maps.append(
            {
                "aug1": aug,
                "s1_idx": meta1[k]["idx"], "s1_valp": meta1[k]["valp"],
                "s1_s8": meta1[k]["s8"], "s1_hd": meta1[k]["hd"],
                "s2_idx": meta2[k]["idx"], "s2_valp": meta2[k]["valp"],
                "s2_s8": meta2[k]["s8"], "s2_hd": meta2[k]["hd"],
            }
        )
    return sched1, sched2, in_maps


def assemble_output(cfg: Config, results):
    out = np.zeros((cfg.n_nodes, cfg.d), dtype=np.float32)
    for k, r in enumerate(results):
        lo = k * cfg.rpc
        hi = min(lo + cfg.rpc, cfg.n_nodes)
        if hi > lo:
            out[lo:hi] = r["out"][0 : hi - lo]
    return out


_CACHE = {}


def kernel(**inputs):
    import concourse.bass_utils as bass_utils

    cfg = Config()
    sched1, sched2, in_maps = prepare(cfg, inputs)
    key = (
        sched1.n_tiles, sched2.n_tiles,
        tuple(sched1.T.ravel()), tuple(sched2.T.ravel()),
    )
    nc = _CACHE.get(key)
    if nc is None:
        nc = build_kernel(cfg, sched1, sched2)
        _CACHE[key] = nc
    res = bass_utils.run_bass_kernel_spmd(
        nc, in_maps, core_ids=list(range(cfg.n_cores)), trace=False
    )
    out = assemble_output(cfg, res.results)
    return out.astype(np.float32, copy=False)


# revision 10
# speedup vs baseline: 2.2654x; 1.0392x over previous
"""Trainium2 Bass kernel for nn_DirectedHyperConvLayer (GNN message passing).

Self-contained: accepts FULL inputs, shards across 8 NeuronCores internally,
returns the FULL [50000, 64] float32 output.

Sharding: each core owns a contiguous block of destination rows; the host
routes/sorts edges by destination row, pads them into 128-edge tiles grouped
by 128-row destination windows, and splits each window's edges into low/high
passes so source indices fit dma_gather's int16 limit. On device, combined
[raw_bf16 | hat_bf16] rows are fetched with batched dma_gather calls; the
per-edge cosine dot hat_dst . hat_src runs on the vector engine against a
host-expanded destination-hat stream; the weight wv = (21 + dot) * (0.05*val)
scales raw gathered rows into messages, which a host-supplied f8 one-hot
(s8, [edge, dest-row]) segment-sums into PSUM per window through the tensor
engine. Stage 1 results (raw bf16) are AllGathered and merged into the
gather table's raw half for stage 2. Scatter matmuls for call c are emitted
during call c+1 so the tensor engine's in-order queue never stalls behind
the dot/weight chain of the current call.
"""


from dataclasses import dataclass

import numpy as np
import ml_dtypes

import concourse.bass as bass
import concourse.bacc as bacc
import concourse.mybir as mybir
import concourse.tile as tile

F32 = mybir.dt.float32
BF16 = mybir.dt.bfloat16
F8 = mybir.dt.float8e4
I16 = mybir.dt.int16
NP_F8 = mybir.dt.np(F8)
NP_BF16 = ml_dtypes.bfloat16
P = 128
TB = 8  # tiles per gather call / compute batch (1024 idx)


@dataclass
class Config:
    n_nodes: int = 50000
    d: int = 64
    n_cores: int = 8
    rpc: int = 6272  # rows per core (multiple of 128)
    split: int = 32768  # low/high gather split (<= 32768)

    @property
    def nw(self):
        return self.rpc // P

    @property
    def n_pad(self):
        return self.n_cores * self.rpc


@dataclass
class StageSched:
    T: np.ndarray  # [2, nw] tiles per (pass, window)
    n_tiles: tuple  # (low, high) tile counts (each % TB == 0)

    @property
    def total_tiles(self):
        return int(self.n_tiles[0] + self.n_tiles[1])

    def tile_windows(self):
        """list over global tile index -> (pass, w, j_in_window, first, last)"""
        out = []
        for p in range(2):
            for w in range(self.T.shape[1]):
                Tw = int(self.T[p, w])
                for j in range(Tw):
                    out.append((p, w, j, j == 0, j == Tw - 1))
        return out


def make_hat(embs):
    """[N, d] f32 -> (raw bf16, hat bf16)."""
    nrm = np.sqrt((embs.astype(np.float64) ** 2).sum(-1))
    nrm = np.maximum(nrm, 1e-8)
    hat = (embs / nrm[:, None].astype(np.float32)).astype(NP_BF16)
    return embs.astype(NP_BF16), hat


def route_edges(cfg: Config, edge_index, edge_val, hat):
    """Returns (sched, per_core list of dicts with idx/valp/s8/hd)."""
    r0 = np.asarray(edge_index[0], dtype=np.int64)
    r1 = np.asarray(edge_index[1], dtype=np.int64)
    val = np.asarray(edge_val, dtype=np.float32)
    E = r0.shape[0]
    nc_, nw = cfg.n_cores, cfg.nw

    k = r0 // cfg.rpc
    w = (r0 % cfg.rpc) // P
    dloc = r0 % P
    hi = (r1 >= cfg.split).astype(np.int64)
    gid = (k * 2 + hi) * nw + w

    counts = np.bincount(gid, minlength=nc_ * 2 * nw).reshape(nc_, 2, nw)
    T = np.ceil(counts.max(axis=0) / P).astype(np.int64)  # [2, nw]
    # pad each pass's tile total to a multiple of TB
    for pss in range(2):
        T[pss, nw - 1] += (-int(T[pss].sum())) % TB
    nt_low, nt_high = int(T[0].sum()), int(T[1].sum())
    n_tiles = nt_low + nt_high
    # global tile base per (pass, w)
    tbase = np.zeros((2, nw), dtype=np.int64)
    tbase[0] = np.cumsum(T[0]) - T[0]
    tbase[1] = nt_low + np.cumsum(T[1]) - T[1]

    # slot within (k, hi, w) group
    order = np.argsort(gid, kind="stable")
    sorted_gid = gid[order]
    starts = np.searchsorted(sorted_gid, np.arange(nc_ * 2 * nw))
    ranks = np.empty(E, dtype=np.int64)
    ranks[order] = np.arange(E) - starts[sorted_gid]

    tile_g = tbase[hi, w] + ranks // P  # global tile per edge
    pos = ranks % P
    idx_val = (r1 - hi * cfg.split).astype(np.int16)

    ncalls = n_tiles // TB
    cw = TB * P // 16
    per_core = []
    for kk in range(nc_):
        m = k == kk
        tg, pg = tile_g[m], pos[m]
        slots = tg * P + pg
        idx_flat = np.zeros(n_tiles * P, dtype=np.int16)
        valp_flat = np.zeros(n_tiles * P, dtype=np.float32)
        idx_flat[slots] = idx_val[m]
        valp_flat[slots] = 0.05 * val[m]
        # idx16 wrapped: [128, ncalls*(TB*P//16)]
        iw = idx_flat.reshape(ncalls, cw, 16)
        iw = np.transpose(iw, (2, 0, 1)).reshape(16, ncalls * cw)
        idx16 = np.tile(iw, (8, 1))
        # valp [128, n_tiles] (partition p, tile t)
        valp2d = valp_flat.reshape(n_tiles, P).T.copy()
        # s8 [128(e), n_tiles*128(d)] f8 one-hot
        dl = dloc[m]
        s8 = np.zeros((P, n_tiles * P), dtype=NP_F8)
        s8[pg, tg * P + dl] = NP_F8(1.0)
        # hd [128(e), n_tiles*64] bf16: dest-row hat per edge slot
        hd = np.zeros((P, n_tiles, cfg.d), dtype=NP_BF16)
        hd[pg, tg] = hat[r0[m]]
        per_core.append(
            {
                "idx": idx16,
                "valp": valp2d,
                "s8": s8,
                "hd": hd.reshape(P, n_tiles * cfg.d),
            }
        )
    return StageSched(T=T, n_tiles=(nt_low, nt_high)), per_core


def _emit_stage(
    tc, cfg, sched: StageSched, pools, table, idx_t, valp_t, s8_dram, hd_dram,
    out_close,
):
    """Emit one spmm stage. out_close(w, psum_ap, acc_ap, has_low) writes the
    finished window."""
    nc = tc.nc
    nw = cfg.nw
    d = cfg.d
    gp, paccp, dvep, msgp, accp, s8p, hdp = (
        pools["g"], pools["pacc"], pools["dve"], pools["msg"], pools["acc"],
        pools["s8"], pools["hd"],
    )

    acc = accp.tile([P, nw * d], F32, tag="acc")
    nc.vector.memset(acc[:], 0.0)

    tw = sched.tile_windows()
    n_tiles = sched.total_tiles
    assert n_tiles % TB == 0
    ncalls = n_tiles // TB
    nt_low = sched.n_tiles[0]
    cw = TB * P // 16  # idx cols per call

    # s8 one-hot tiles, streamed per (pass, w)
    s8_tiles = {}
    st0 = 0
    for pss in range(2):
        for w in range(nw):
            Tw = int(sched.T[pss, w])
            if Tw == 0:
                continue
            s8t = s8p.tile([P, Tw * P], F8, tag="s8", name="s8")
            nc.scalar.dma_start(s8t[:], s8_dram[:, st0 * P : (st0 + Tw) * P])
            s8_tiles[(pss, w)] = (s8t, st0)
            st0 += Tw

    win_psum = {}

    def emit_scatter(c, msgs):
        t0 = c * TB
        for sl in range(TB):
            t = t0 + sl
            pss_t, w, j, first, last = tw[t]
            key = (pss_t, w)
            if key not in win_psum:
                win_psum[key] = paccp.tile(
                    [P, d], F32, space="PSUM", tag="pacc", name="pacc"
                )
            pw = win_psum[key]
            s8t, st_t0 = s8_tiles[key]
            jj = t - st_t0
            nc.tensor.matmul(
                out=pw[:],
                lhsT=s8t[:, jj * P : (jj + 1) * P],
                rhs=msgs[:, sl, :],
                start=first,
                stop=last,
            )
            if last:
                if pss_t == 0:
                    nc.scalar.copy(out=acc[:, w * d : (w + 1) * d], in_=pw[:])
                else:
                    has_low = sched.T[0, w] > 0
                    out_close(w, pw[:], acc[:, w * d : (w + 1) * d], has_low)
                del win_psum[key]

    pend = None  # (c, msgs) awaiting scatter
    for c in range(ncalls):
        pss = 0 if c * TB < nt_low else 1
        tab = table[0 : cfg.split, :] if pss == 0 else table[cfg.split : cfg.n_pad, :]
        g = gp.tile([P, TB, 2 * d], BF16, tag="g")
        nc.gpsimd.dma_gather(
            out_ap=g[:],
            in_ap=tab,
            idxs_ap=idx_t[:, c * cw : (c + 1) * cw],
            num_idxs=TB * P,
            num_idxs_reg=TB * P,
            elem_size=2 * d,
            queue_num=c % 4,
            single_packet=True,
        )
        t0 = c * TB
        hd = hdp.tile([P, TB, d], BF16, tag="hd")
        nc.scalar.dma_start(hd[:], hd_dram[:, t0 * d : (t0 + TB) * d])
        # dot per edge: dot1[e] = 21 + sum_f hd[e,f] * ghat[e,f]
        prod = dvep.tile([P, TB, d], BF16, tag="prod")
        nc.vector.tensor_tensor(
            out=prod[:], in0=hd[:], in1=g[:, :, d : 2 * d],
            op=mybir.AluOpType.mult,
        )
        dot1 = dvep.tile([P, TB], F32, tag="dot1")
        nc.vector.tensor_reduce(
            out=dot1[:], in_=prod[:], op=mybir.AluOpType.add,
            axis=mybir.AxisListType.X,
        )
        nc.vector.tensor_scalar_add(dot1[:], dot1[:], 21.0)
        # wv = dot1 * (0.05*val)  (== val * (1.05 + 0.05*sim_dot))
        wv = dvep.tile([P, TB], F32, tag="wv")
        nc.vector.tensor_tensor(
            out=wv[:], in0=dot1[:], in1=valp_t[:, t0 : t0 + TB],
            op=mybir.AluOpType.mult,
        )
        # weighted messages: msgs[e, sl, f] = wv[e, sl] * g_raw[e, sl, f]
        wvb = dvep.tile([P, TB], BF16, tag="wvb")
        nc.vector.tensor_copy(out=wvb[:], in_=wv[:])
        msgs = msgp.tile([P, TB, d], BF16, tag="msgs")
        nc.vector.tensor_tensor(
            out=msgs[:], in0=g[:, :, 0:d],
            in1=wvb[:].to_broadcast([P, TB, d]),
            op=mybir.AluOpType.mult,
        )
        if pend is not None:
            emit_scatter(*pend)
        pend = (c, msgs)
    emit_scatter(*pend)
    # windows with no high-pass tiles: close from acc only
    for w in range(nw):
        if sched.T[1, w] == 0:
            out_close(w, None, acc[:, w * d : (w + 1) * d], sched.T[0, w] > 0)


def build_kernel(cfg: Config, sched1: StageSched, sched2: StageSched):
    nc = bacc.Bacc(
        "TRN2",
        target_bir_lowering=False,
        debug=False,
        enable_asserts=False,
        num_devices=cfg.n_cores,
        num_swdge_queues=4,
    )
    d = cfg.d
    aug1 = nc.dram_tensor("aug1", [cfg.n_pad, 2 * d], BF16, kind="ExternalInput")
    sio = {}
    for s, sch in (("s1", sched1), ("s2", sched2)):
        nt = sch.total_tiles
        sio[s] = {
            "idx": nc.dram_tensor(
                f"{s}_idx", [P, (nt // TB) * (TB * P // 16)], I16,
                kind="ExternalInput",
            ),
            "valp": nc.dram_tensor(f"{s}_valp", [P, nt], F32, kind="ExternalInput"),
            "s8": nc.dram_tensor(f"{s}_s8", [P, nt * P], F8, kind="ExternalInput"),
            "hd": nc.dram_tensor(f"{s}_hd", [P, nt * d], BF16, kind="ExternalInput"),
        }
    out = nc.dram_tensor("out", [cfg.rpc, d], F32, kind="ExternalOutput")
    aug2 = nc.dram_tensor("aug2", [cfg.n_pad, 2 * d], BF16)
    msgtar_local = nc.dram_tensor("mt_loc", [cfg.rpc, d], BF16)
    msgtar_full = nc.dram_tensor("mt_full", [cfg.n_pad, d], BF16)

    with tile.TileContext(nc) as tc:
        pools = {
            "g": tc.alloc_tile_pool(name="g", bufs=16),
            "pacc": tc.alloc_tile_pool(name="pacc", bufs=6, space="PSUM"),
            "s8": tc.alloc_tile_pool(name="s8p", bufs=6),
            "hd": tc.alloc_tile_pool(name="hdp", bufs=12),
            "dve": tc.alloc_tile_pool(name="dve", bufs=10),
            "msg": tc.alloc_tile_pool(name="msg", bufs=10),
            "acc": tc.alloc_tile_pool(name="accp", bufs=1),
            "io": tc.alloc_tile_pool(name="iop", bufs=1),
            "wout": tc.alloc_tile_pool(name="wout", bufs=4),
        }

        def load_stage_meta(s):
            io = pools["io"]
            t = sio[s]
            nt = t["valp"].shape[1]
            idx_t = io.tile([P, t["idx"].shape[1]], I16, tag=f"{s}i", name=f"{s}i")
            nc.sync.dma_start(idx_t[:], t["idx"][:, :])
            valp_t = io.tile([P, nt], F32, tag=f"{s}v", name=f"{s}v")
            nc.sync.dma_start(valp_t[:], t["valp"][:, :])
            return idx_t, valp_t, t["s8"], t["hd"]

        # ---------------- stage 1 (tar) ----------------
        idx1, valp1, s81, hd1 = load_stage_meta("s1")

        def close1(w, psum_ap, acc_ap, has_low):
            ot = pools["wout"].tile([P, d], BF16, tag="wo1")
            if psum_ap is None:
                nc.scalar.copy(out=ot[:], in_=acc_ap)
            elif has_low:
                nc.vector.tensor_tensor(
                    out=ot[:], in0=acc_ap, in1=psum_ap, op=mybir.AluOpType.add
                )
            else:
                nc.scalar.copy(out=ot[:], in_=psum_ap)
            nc.sync.dma_start(
                out=msgtar_local[w * P : (w + 1) * P, :], in_=ot[:]
            )

        _emit_stage(
            tc, cfg, sched1, pools, aug1[:], idx1, valp1, s81[:, :], hd1[:, :],
            close1,
        )

        # stage-2 table hat half (overlaps stage-1 tail / collective)
        nc.scalar.dma_start(out=aug2[:, d : 2 * d], in_=aug1[:, d : 2 * d])

        # ---------------- allgather (raw msg_tar only) ----------------
        nc.gpsimd.collective_compute(
            "AllGather",
            mybir.AluOpType.bypass,
            replica_groups=[list(range(cfg.n_cores))],
            ins=[msgtar_local[:].opt()],
            outs=[msgtar_full[:].opt()],
        )
        # merge gathered raw into aug2's raw half (hat half copied earlier)
        nc.sync.dma_start(out=aug2[:, 0:d], in_=msgtar_full[:, :])

        # ---------------- stage 2 (src) ----------------
        idx2, valp2, s82, hd2 = load_stage_meta("s2")

        def close2(w, psum_ap, acc_ap, has_low):
            ot = pools["wout"].tile([P, d], F32, tag="wo2")
            if psum_ap is None:
                nc.scalar.copy(out=ot[:], in_=acc_ap)
            elif has_low:
                nc.vector.tensor_tensor(
                    out=ot[:], in0=acc_ap, in1=psum_ap, op=mybir.AluOpType.add
                )
            else:
                nc.scalar.copy(out=ot[:], in_=psum_ap)
            nc.sync.dma_start(out=out[w * P : (w + 1) * P, :], in_=ot[:])

        _emit_stage(
            tc, cfg, sched2, pools, aug2[:], idx2, valp2, s82[:, :], hd2[:, :],
            close2,
        )

        for p in reversed(list(pools.values())):
            p.release()

    nc.compile()
    return nc


def prepare(cfg: Config, inputs):
    """inputs: dict with pois_embs, src_edge_index, src_edge_val, tar_*."""
    embs = np.asarray(inputs["pois_embs"], dtype=np.float32)
    raw, hat = make_hat(embs)
    aug = np.zeros((cfg.n_pad, 2 * cfg.d), dtype=NP_BF16)
    aug[: cfg.n_nodes, : cfg.d] = raw
    aug[: cfg.n_nodes, cfg.d :] = hat
    sched1, meta1 = route_edges(
        cfg, inputs["tar_edge_index"], inputs["tar_edge_val"], hat
    )
    sched2, meta2 = route_edges(
        cfg, inputs["src_edge_index"], inputs["src_edge_val"], hat
    )
    in_maps = []
    for k in range(cfg.n_cores):
        in_maps.append(
            {
                "aug1": aug,
                "s1_idx": meta1[k]["idx"], "s1_valp": meta1[k]["valp"],
                "s1_s8": meta1[k]["s8"], "s1_hd": meta1[k]["hd"],
                "s2_idx": meta2[k]["idx"], "s2_valp": meta2[k]["valp"],
                "s2_s8": meta2[k]["s8"], "s2_hd": meta2[k]["hd"],
            }
        )
    return sched1, sched2, in_maps


def assemble_output(cfg: Config, results):
    out = np.zeros((cfg.n_nodes, cfg.d), dtype=np.float32)
    for k, r in enumerate(results):
        lo = k * cfg.rpc
        hi = min(lo + cfg.rpc, cfg.n_nodes)
        if hi > lo:
            out[lo:hi] = r["out"][0 : hi - lo]
    return out


_CACHE = {}


def kernel(**inputs):
    import concourse.bass_utils as bass_utils

    cfg = Config()
    sched1, sched2, in_maps = prepare(cfg, inputs)
    key = (
        sched1.n_tiles, sched2.n_tiles,
        tuple(sched1.T.ravel()), tuple(sched2.T.ravel()),
    )
    nc = _CACHE.get(key)
    if nc is None:
        nc = build_kernel(cfg, sched1, sched2)
        _CACHE[key] = nc
    res = bass_utils.run_bass_kernel_spmd(
        nc, in_maps, core_ids=list(range(cfg.n_cores)), trace=False
    )
    out = assemble_output(cfg, res.results)
    return out.astype(np.float32, copy=False)


# revision 11
# speedup vs baseline: 2.2827x; 1.0076x over previous
"""Trainium2 Bass kernel for nn_DirectedHyperConvLayer (GNN message passing).

Self-contained: accepts FULL inputs, shards across 8 NeuronCores internally,
returns the FULL [50000, 64] float32 output.

Sharding: each core owns a contiguous block of destination rows; the host
routes/sorts edges by destination row, pads them into 128-edge tiles grouped
by 128-row destination windows, and splits each window's edges into low/high
passes so source indices fit dma_gather's int16 limit. On device, combined
[raw_bf16 | hat_bf16] rows are fetched with batched dma_gather calls; the
per-edge cosine dot hat_dst . hat_src runs on the vector engine against a
host-expanded destination-hat stream; the weight wv = (21 + dot) * (0.05*val)
scales raw gathered rows into messages, which a host-supplied f8 one-hot
(s8, [edge, dest-row]) segment-sums into PSUM per window through the tensor
engine. Stage 1 results (raw bf16) are AllGathered and merged into the
gather table's raw half for stage 2. Scatter matmuls for call c are emitted
during call c+1 so the tensor engine's in-order queue never stalls behind
the dot/weight chain of the current call.
"""


from dataclasses import dataclass

import numpy as np
import ml_dtypes

import concourse.bass as bass
import concourse.bacc as bacc
import concourse.mybir as mybir
import concourse.tile as tile

F32 = mybir.dt.float32
BF16 = mybir.dt.bfloat16
F8 = mybir.dt.float8e4
I16 = mybir.dt.int16
NP_F8 = mybir.dt.np(F8)
NP_BF16 = ml_dtypes.bfloat16
P = 128
TB = 8  # tiles per gather call / compute batch (1024 idx)
HDC = 8  # hd-stream chunk, in calls


@dataclass
class Config:
    n_nodes: int = 50000
    d: int = 64
    n_cores: int = 8
    rpc: int = 6272  # rows per core (multiple of 128)
    split: int = 32768  # low/high gather split (<= 32768)

    @property
    def nw(self):
        return self.rpc // P

    @property
    def n_pad(self):
        return self.n_cores * self.rpc


@dataclass
class StageSched:
    T: np.ndarray  # [2, nw] tiles per (pass, window)
    n_tiles: tuple  # (low, high) tile counts (each % TB == 0)

    @property
    def total_tiles(self):
        return int(self.n_tiles[0] + self.n_tiles[1])

    def tile_windows(self):
        """list over global tile index -> (pass, w, j_in_window, first, last)"""
        out = []
        for p in range(2):
            for w in range(self.T.shape[1]):
                Tw = int(self.T[p, w])
                for j in range(Tw):
                    out.append((p, w, j, j == 0, j == Tw - 1))
        return out


def make_hat(embs):
    """[N, d] f32 -> (raw bf16, hat bf16)."""
    nrm = np.sqrt((embs.astype(np.float64) ** 2).sum(-1))
    nrm = np.maximum(nrm, 1e-8)
    hat = (embs / nrm[:, None].astype(np.float32)).astype(NP_BF16)
    return embs.astype(NP_BF16), hat


def route_edges(cfg: Config, edge_index, edge_val, hat):
    """Returns (sched, per_core list of dicts with idx/valp/s8/hd)."""
    r0 = np.asarray(edge_index[0], dtype=np.int64)
    r1 = np.asarray(edge_index[1], dtype=np.int64)
    val = np.asarray(edge_val, dtype=np.float32)
    E = r0.shape[0]
    nc_, nw = cfg.n_cores, cfg.nw

    k = r0 // cfg.rpc
    w = (r0 % cfg.rpc) // P
    dloc = r0 % P
    hi = (r1 >= cfg.split).astype(np.int64)
    gid = (k * 2 + hi) * nw + w

    counts = np.bincount(gid, minlength=nc_ * 2 * nw).reshape(nc_, 2, nw)
    T = np.ceil(counts.max(axis=0) / P).astype(np.int64)  # [2, nw]
    # pad each pass's tile total to a multiple of TB
    for pss in range(2):
        T[pss, nw - 1] += (-int(T[pss].sum())) % TB
    nt_low, nt_high = int(T[0].sum()), int(T[1].sum())
    n_tiles = nt_low + nt_high
    # global tile base per (pass, w)
    tbase = np.zeros((2, nw), dtype=np.int64)
    tbase[0] = np.cumsum(T[0]) - T[0]
    tbase[1] = nt_low + np.cumsum(T[1]) - T[1]

    # slot within (k, hi, w) group
    order = np.argsort(gid, kind="stable")
    sorted_gid = gid[order]
    starts = np.searchsorted(sorted_gid, np.arange(nc_ * 2 * nw))
    ranks = np.empty(E, dtype=np.int64)
    ranks[order] = np.arange(E) - starts[sorted_gid]

    tile_g = tbase[hi, w] + ranks // P  # global tile per edge
    pos = ranks % P
    idx_val = (r1 - hi * cfg.split).astype(np.int16)

    ncalls = n_tiles // TB
    cw = TB * P // 16
    per_core = []
    for kk in range(nc_):
        m = k == kk
        tg, pg = tile_g[m], pos[m]
        slots = tg * P + pg
        idx_flat = np.zeros(n_tiles * P, dtype=np.int16)
        valp_flat = np.zeros(n_tiles * P, dtype=np.float32)
        idx_flat[slots] = idx_val[m]
        valp_flat[slots] = 0.05 * val[m]
        # idx16 wrapped: [128, ncalls*(TB*P//16)]
        iw = idx_flat.reshape(ncalls, cw, 16)
        iw = np.transpose(iw, (2, 0, 1)).reshape(16, ncalls * cw)
        idx16 = np.tile(iw, (8, 1))
        # valp [128, n_tiles] (partition p, tile t)
        valp2d = valp_flat.reshape(n_tiles, P).T.copy()
        # s8 [128(e), n_tiles*128(d)] f8 one-hot
        dl = dloc[m]
        s8 = np.zeros((P, n_tiles * P), dtype=NP_F8)
        s8[pg, tg * P + dl] = NP_F8(1.0)
        # hd [128(e), n_tiles*64] bf16: dest-row hat per edge slot
        hd = np.zeros((P, n_tiles, cfg.d), dtype=NP_BF16)
        hd[pg, tg] = hat[r0[m]]
        per_core.append(
            {
                "idx": idx16,
                "valp": valp2d,
                "s8": s8,
                "hd": hd.reshape(P, n_tiles * cfg.d),
            }
        )
    return StageSched(T=T, n_tiles=(nt_low, nt_high)), per_core


def _emit_stage(
    tc, cfg, sched: StageSched, pools, table, idx_t, valp_t, s8_dram, hd_dram,
    out_close,
):
    """Emit one spmm stage. out_close(w, psum_ap, acc_ap, has_low) writes the
    finished window."""
    nc = tc.nc
    nw = cfg.nw
    d = cfg.d
    gp, paccp, dvep, msgp, accp, s8p, hdp = (
        pools["g"], pools["pacc"], pools["dve"], pools["msg"], pools["acc"],
        pools["s8"], pools["hd"],
    )

    acc = accp.tile([P, nw * d], F32, tag="acc")
    nc.vector.memset(acc[:], 0.0)

    tw = sched.tile_windows()
    n_tiles = sched.total_tiles
    assert n_tiles % TB == 0
    ncalls = n_tiles // TB
    nt_low = sched.n_tiles[0]
    cw = TB * P // 16  # idx cols per call

    # s8 one-hot tiles, streamed per (pass, w)
    s8_tiles = {}
    st0 = 0
    for pss in range(2):
        for w in range(nw):
            Tw = int(sched.T[pss, w])
            if Tw == 0:
                continue
            s8t = s8p.tile([P, Tw * P], F8, tag="s8", name="s8")
            nc.scalar.dma_start(s8t[:], s8_dram[:, st0 * P : (st0 + Tw) * P])
            s8_tiles[(pss, w)] = (s8t, st0)
            st0 += Tw

    win_psum = {}

    def emit_scatter(c, msgs):
        t0 = c * TB
        for sl in range(TB):
            t = t0 + sl
            pss_t, w, j, first, last = tw[t]
            key = (pss_t, w)
            if key not in win_psum:
                win_psum[key] = paccp.tile(
                    [P, d], F32, space="PSUM", tag="pacc", name="pacc"
                )
            pw = win_psum[key]
            s8t, st_t0 = s8_tiles[key]
            jj = t - st_t0
            nc.tensor.matmul(
                out=pw[:],
                lhsT=s8t[:, jj * P : (jj + 1) * P],
                rhs=msgs[:, sl, :],
                start=first,
                stop=last,
            )
            if last:
                if pss_t == 0:
                    nc.scalar.copy(out=acc[:, w * d : (w + 1) * d], in_=pw[:])
                else:
                    has_low = sched.T[0, w] > 0
                    out_close(w, pw[:], acc[:, w * d : (w + 1) * d], has_low)
                del win_psum[key]

    pend = None  # (c, msgs) awaiting scatter
    for c in range(ncalls):
        pss = 0 if c * TB < nt_low else 1
        tab = table[0 : cfg.split, :] if pss == 0 else table[cfg.split : cfg.n_pad, :]
        g = gp.tile([P, TB, 2 * d], BF16, tag="g")
        nc.gpsimd.dma_gather(
            out_ap=g[:],
            in_ap=tab,
            idxs_ap=idx_t[:, c * cw : (c + 1) * cw],
            num_idxs=TB * P,
            num_idxs_reg=TB * P,
            elem_size=2 * d,
            queue_num=c % 4,
            single_packet=True,
        )
        t0 = c * TB
        if c % HDC == 0:
            hdc = hdp.tile([P, HDC * TB, d], BF16, tag="hd")
            nchunk = min(HDC * TB, n_tiles - t0)
            nc.scalar.dma_start(
                hdc[:, 0:nchunk, :], hd_dram[:, t0 * d : (t0 + nchunk) * d]
            )
        hd = hdc[:, (c % HDC) * TB : (c % HDC) * TB + TB, :]
        # dot per edge: dot1[e] = 21 + sum_f hd[e,f] * ghat[e,f]
        prod = dvep.tile([P, TB, d], BF16, tag="prod")
        nc.vector.tensor_tensor(
            out=prod[:], in0=hd, in1=g[:, :, d : 2 * d],
            op=mybir.AluOpType.mult,
        )
        dot1 = dvep.tile([P, TB], F32, tag="dot1")
        nc.vector.tensor_reduce(
            out=dot1[:], in_=prod[:], op=mybir.AluOpType.add,
            axis=mybir.AxisListType.X,
        )
        nc.vector.tensor_scalar_add(dot1[:], dot1[:], 21.0)
        # wv = dot1 * (0.05*val)  (== val * (1.05 + 0.05*sim_dot))
        wv = dvep.tile([P, TB], F32, tag="wv")
        nc.vector.tensor_tensor(
            out=wv[:], in0=dot1[:], in1=valp_t[:, t0 : t0 + TB],
            op=mybir.AluOpType.mult,
        )
        # weighted messages: msgs[e, sl, f] = wv[e, sl] * g_raw[e, sl, f]
        wvb = dvep.tile([P, TB], BF16, tag="wvb")
        nc.vector.tensor_copy(out=wvb[:], in_=wv[:])
        msgs = msgp.tile([P, TB, d], BF16, tag="msgs")
        nc.vector.tensor_tensor(
            out=msgs[:], in0=g[:, :, 0:d],
            in1=wvb[:].to_broadcast([P, TB, d]),
            op=mybir.AluOpType.mult,
        )
        if pend is not None:
            emit_scatter(*pend)
        pend = (c, msgs)
    emit_scatter(*pend)
    # windows with no high-pass tiles: close from acc only
    for w in range(nw):
        if sched.T[1, w] == 0:
            out_close(w, None, acc[:, w * d : (w + 1) * d], sched.T[0, w] > 0)


def build_kernel(cfg: Config, sched1: StageSched, sched2: StageSched):
    nc = bacc.Bacc(
        "TRN2",
        target_bir_lowering=False,
        debug=False,
        enable_asserts=False,
        num_devices=cfg.n_cores,
        num_swdge_queues=4,
        dynamic_dma_scratch_size=32768,
    )
    d = cfg.d
    aug1 = nc.dram_tensor("aug1", [cfg.n_pad, 2 * d], BF16, kind="ExternalInput")
    sio = {}
    for s, sch in (("s1", sched1), ("s2", sched2)):
        nt = sch.total_tiles
        sio[s] = {
            "idx": nc.dram_tensor(
                f"{s}_idx", [P, (nt // TB) * (TB * P // 16)], I16,
                kind="ExternalInput",
            ),
            "valp": nc.dram_tensor(f"{s}_valp", [P, nt], F32, kind="ExternalInput"),
            "s8": nc.dram_tensor(f"{s}_s8", [P, nt * P], F8, kind="ExternalInput"),
            "hd": nc.dram_tensor(f"{s}_hd", [P, nt * d], BF16, kind="ExternalInput"),
        }
    out = nc.dram_tensor("out", [cfg.rpc, d], F32, kind="ExternalOutput")
    aug2 = nc.dram_tensor("aug2", [cfg.n_pad, 2 * d], BF16)
    msgtar_local = nc.dram_tensor("mt_loc", [cfg.rpc, d], BF16)
    msgtar_full = nc.dram_tensor("mt_full", [cfg.n_pad, d], BF16)

    with tile.TileContext(nc) as tc:
        pools = {
            "g": tc.alloc_tile_pool(name="g", bufs=16),
            "pacc": tc.alloc_tile_pool(name="pacc", bufs=6, space="PSUM"),
            "s8": tc.alloc_tile_pool(name="s8p", bufs=6),
            "hd": tc.alloc_tile_pool(name="hdp", bufs=3),
            "dve": tc.alloc_tile_pool(name="dve", bufs=10),
            "msg": tc.alloc_tile_pool(name="msg", bufs=10),
            "acc": tc.alloc_tile_pool(name="accp", bufs=1),
            "io": tc.alloc_tile_pool(name="iop", bufs=1),
            "wout": tc.alloc_tile_pool(name="wout", bufs=4),
        }

        def load_stage_meta(s):
            io = pools["io"]
            t = sio[s]
            nt = t["valp"].shape[1]
            idx_t = io.tile([P, t["idx"].shape[1]], I16, tag=f"{s}i", name=f"{s}i")
            nc.sync.dma_start(idx_t[:], t["idx"][:, :])
            valp_t = io.tile([P, nt], F32, tag=f"{s}v", name=f"{s}v")
            nc.sync.dma_start(valp_t[:], t["valp"][:, :])
            return idx_t, valp_t, t["s8"], t["hd"]

        # ---------------- stage 1 (tar) ----------------
        idx1, valp1, s81, hd1 = load_stage_meta("s1")

        def close1(w, psum_ap, acc_ap, has_low):
            ot = pools["wout"].tile([P, d], BF16, tag="wo1")
            if psum_ap is None:
                nc.scalar.copy(out=ot[:], in_=acc_ap)
            elif has_low:
                nc.vector.tensor_tensor(
                    out=ot[:], in0=acc_ap, in1=psum_ap, op=mybir.AluOpType.add
                )
            else:
                nc.scalar.copy(out=ot[:], in_=psum_ap)
            nc.sync.dma_start(
                out=msgtar_local[w * P : (w + 1) * P, :], in_=ot[:]
            )

        _emit_stage(
            tc, cfg, sched1, pools, aug1[:], idx1, valp1, s81[:, :], hd1[:, :],
            close1,
        )

        # stage-2 table hat half (overlaps stage-1 tail / collective)
        nc.scalar.dma_start(out=aug2[:, d : 2 * d], in_=aug1[:, d : 2 * d])

        # ---------------- allgather (raw msg_tar only) ----------------
        nc.gpsimd.collective_compute(
            "AllGather",
            mybir.AluOpType.bypass,
            replica_groups=[list(range(cfg.n_cores))],
            ins=[msgtar_local[:].opt()],
            outs=[msgtar_full[:].opt()],
        )
        # merge gathered raw into aug2's raw half (hat half copied earlier)
        nc.sync.dma_start(out=aug2[:, 0:d], in_=msgtar_full[:, :])

        # ---------------- stage 2 (src) ----------------
        idx2, valp2, s82, hd2 = load_stage_meta("s2")

        def close2(w, psum_ap, acc_ap, has_low):
            ot = pools["wout"].tile([P, d], F32, tag="wo2")
            if psum_ap is None:
                nc.scalar.copy(out=ot[:], in_=acc_ap)
            elif has_low:
                nc.vector.tensor_tensor(
                    out=ot[:], in0=acc_ap, in1=psum_ap, op=mybir.AluOpType.add
                )
            else:
                nc.scalar.copy(out=ot[:], in_=psum_ap)
            nc.sync.dma_start(out=out[w * P : (w + 1) * P, :], in_=ot[:])

        _emit_stage(
            tc, cfg, sched2, pools, aug2[:], idx2, valp2, s82[:, :], hd2[:, :],
            close2,
        )

        for p in reversed(list(pools.values())):
            p.release()

    nc.compile()
    return nc


def prepare(cfg: Config, inputs):
    """inputs: dict with pois_embs, src_edge_index, src_edge_val, tar_*."""
    embs = np.asarray(inputs["pois_embs"], dtype=np.float32)
    raw, hat = make_hat(embs)
    aug = np.zeros((cfg.n_pad, 2 * cfg.d), dtype=NP_BF16)
    aug[: cfg.n_nodes, : cfg.d] = raw
    aug[: cfg.n_nodes, cfg.d :] = hat
    sched1, meta1 = route_edges(
        cfg, inputs["tar_edge_index"], inputs["tar_edge_val"], hat
    )
    sched2, meta2 = route_edges(
        cfg, inputs["src_edge_index"], inputs["src_edge_val"], hat
    )
    in_maps = []
    for k in range(cfg.n_cores):
        in_maps.append(
            {
                "aug1": aug,
                "s1_idx": meta1[k]["idx"], "s1_valp": meta1[k]["valp"],
                "s1_s8": meta1[k]["s8"], "s1_hd": meta1[k]["hd"],
                "s2_idx": meta2[k]["idx"], "s2_valp": meta2[k]["valp"],
                "s2_s8": meta2[k]["s8"], "s2_hd": meta2[k]["hd"],
            }
        )
    return sched1, sched2, in_maps


def assemble_output(cfg: Config, results):
    out = np.zeros((cfg.n_nodes, cfg.d), dtype=np.float32)
    for k, r in enumerate(results):
        lo = k * cfg.rpc
        hi = min(lo + cfg.rpc, cfg.n_nodes)
        if hi > lo:
            out[lo:hi] = r["out"][0 : hi - lo]
    return out


_CACHE = {}


def kernel(**inputs):
    import concourse.bass_utils as bass_utils

    cfg = Config()
    sched1, sched2, in_maps = prepare(cfg, inputs)
    key = (
        sched1.n_tiles, sched2.n_tiles,
        tuple(sched1.T.ravel()), tuple(sched2.T.ravel()),
    )
    nc = _CACHE.get(key)
    if nc is None:
        nc = build_kernel(cfg, sched1, sched2)
        _CACHE[key] = nc
    res = bass_utils.run_bass_kernel_spmd(
        nc, in_maps, core_ids=list(range(cfg.n_cores)), trace=False
    )
    out = assemble_output(cfg, res.results)
    return out.astype(np.float32, copy=False)
